# revision 38
# baseline (speedup 1.0000x reference)
"""Self-contained Trainium2 Bass kernel for the 2-layer Llama3 model.

Sharding: token-sharded compute over 8 cores (core c owns token blocks
{c, 15-c} of each batch, 512 tokens/core), with *minimal host->device
input bytes*: every weight is shipped exactly once, row-sharded by its
contraction dim (1/8 per core), and AllGathered on-device over the
intra-chip links, overlapped with compute. Embedding rows are gathered
on host (2MB/core instead of the 131MB fp32 table). The vocab head is
vocab-sharded: each core keeps its [D, V/8] slice of w_out, final
hidden states are AllGathered (8MB), and each core computes logits for
ALL tokens x its vocab slice, emitted as fp16 (host casts to f32).

Per-core input ~20MB (vs ~257MB replicated), output 32MB fp16.

Device layouts: activations transposed [feature, token] in 128-part
chunks; scores computed transposed [sk, sq]; softmax denominator via a
ones-augmented column on v; RoPE via a +-1 rotation matrix on the PE.
SPMD-uniform program: per-core causality lives in mask *data*.
"""
from contextlib import ExitStack

import numpy as np
import ml_dtypes

import concourse.bass as bass
from concourse.bacc import Bacc
import concourse.mybir as mybir
import concourse.tile as tile
from concourse.bass_utils import run_bass_kernel_spmd

BF16 = ml_dtypes.bfloat16
F32 = mybir.dt.float32
F16 = mybir.dt.float16
BF = mybir.dt.bfloat16

V, D, H, KVH, HD, F, L, B, S = 32000, 1024, 16, 4, 64, 4096, 2, 2, 2048
ROPE_BASE = 500000.0
EPS = 1e-5
SCALE = 1.0 / HD ** 0.5
C = 8            # cores
NB = 16          # 128-token blocks per batch
BS = 128         # block size
OWN = 512        # own tokens per core (2 blocks x 2 batches)
VS = V // C      # vocab shard per core (4000)
NVT = (VS + 511) // 512   # 8 vocab tiles (last is 416 wide)

Exp = mybir.ActivationFunctionType.Exp
Silu = mybir.ActivationFunctionType.Silu
Sqrt = mybir.ActivationFunctionType.Sqrt
Copy = mybir.ActivationFunctionType.Copy
Square = mybir.ActivationFunctionType.Square
MULT = mybir.AluOpType.mult
ADD = mybir.AluOpType.add


def own_blocks(c):
    return [c, NB - 1 - c]


def _colseg(b, t):
    """Own-token column range for (batch b, tile t in {0=low,1=high})."""
    return slice(256 * b + 128 * t, 256 * b + 128 * (t + 1))


# ---------------------------------------------------------------- device ---

def build_nc():
    nc = Bacc()

    x0 = nc.dram_tensor("x0", [8, 128, OWN], F32, kind="ExternalInput")
    cosT = nc.dram_tensor("cosT", [128, OWN], F32, kind="ExternalInput")
    sinT = nc.dram_tensor("sinT", [128, OWN], F32, kind="ExternalInput")
    rmat = nc.dram_tensor("rmat", [128, 128], BF, kind="ExternalInput")
    masks = nc.dram_tensor("masks", [16, 128, 512], BF, kind="ExternalInput")
    # weight shards, row-sharded by contraction dim (1/8 per core):
    # qkv_sh cols: wk 0:256 | wv 256:512 | wq 512:1536
    qkv_sh = nc.dram_tensor("qkv_sh", [L, 128, 1536], BF, kind="ExternalInput")
    wo_sh = nc.dram_tensor("wo_sh", [L, 128, 1024], BF, kind="ExternalInput")
    # ffn_sh cols: 8 groups of [wg 512 | wu 512] (0:8192), then wd
    # as 4 groups of 1024 (F-chunks 4c..4c+3, each [128, D]) at 8192:12288
    ffn_sh = nc.dram_tensor("ffn_sh", [L, 128, 12288], BF, kind="ExternalInput")
    woutc = nc.dram_tensor("woutc", [D, VS], BF, kind="ExternalInput")
    logits = nc.dram_tensor("logits", [C * OWN, VS], F16, kind="ExternalOutput")

    # collective staging (internal) and gathered (Shared) buffers
    qkv_st = nc.dram_tensor("qkv_st", [L, 128, 1536], BF)
    wo_st = nc.dram_tensor("wo_st", [L, 128, 1024], BF)
    ffn_st = nc.dram_tensor("ffn_st", [L, 128, 12288], BF)
    nf_st = nc.dram_tensor("nf_st", [128, 8 * OWN], BF)
    qkv_g = [nc.dram_tensor(f"qkvg{l}", [C, 128, 1536], BF,
                            addr_space="Shared") for l in range(L)]
    wo_g = [nc.dram_tensor(f"wog{l}", [C, 128, 1024], BF,
                           addr_space="Shared") for l in range(L)]
    ffn_g = [nc.dram_tensor(f"ffng{l}", [C, 128, 12288], BF,
                            addr_space="Shared") for l in range(L)]
    nf_g = nc.dram_tensor("nfg", [C, 128, 8 * OWN], BF, addr_space="Shared")
    kvs = [nc.dram_tensor(f"kvs{l}", [B, 2, 256, 256], BF) for l in range(L)]
    kvr = [nc.dram_tensor(f"kvr{l}", [C, B, 2, 256, 256], BF,
                          addr_space="Shared") for l in range(L)]

    def ag(src_ap, dst_ap):
        nc.gpsimd.collective_compute(
            "AllGather", mybir.AluOpType.bypass,
            replica_groups=[list(range(C))],
            ins=[src_ap], outs=[dst_ap])

    with tile.TileContext(nc) as tc, ExitStack() as st:
        npool = st.enter_context(tc.tile_pool(name="npool", bufs=1))
        sbh = st.enter_context(tc.tile_pool(name="sbh", bufs=2))
        psA = st.enter_context(tc.tile_pool(name="psA", bufs=2, space="PSUM"))
        psB = st.enter_context(tc.tile_pool(name="psB", bufs=2, space="PSUM"))
        psM = st.enter_context(tc.tile_pool(name="psM", bufs=3, space="PSUM"))

        with ExitStack() as body:
            const = body.enter_context(tc.tile_pool(name="const", bufs=1))
            resid = body.enter_context(tc.tile_pool(name="resid", bufs=1))
            qpool = body.enter_context(tc.tile_pool(name="qpool", bufs=1))
            apool = body.enter_context(tc.tile_pool(name="apool", bufs=1))
            hpool = body.enter_context(tc.tile_pool(name="hpool", bufs=1))
            sb = body.enter_context(tc.tile_pool(name="sb", bufs=2))
            wbig = body.enter_context(tc.tile_pool(name="wbig", bufs=2))

            # constants FIRST: memsets live on the gpsimd queue, which the
            # collective waits also occupy — emit before any ag()
            ones_col = const.tile([128, 1], BF, tag="ones_col")
            nc.any.memset(ones_col[:], 1.0)
            ones_row = const.tile([1, 128], BF, tag="ones_row")
            nc.any.memset(ones_row[:], 1.0)
            eps_t = const.tile([1, 1], F32, tag="eps")
            nc.any.memset(eps_t[:], EPS)

            # residual stream xT: 8 chunks [128, OWN] f32, resident
            x = [resid.tile([128, OWN], F32, tag=f"x{k}", name=f"x{k}")
                 for k in range(8)]
            for k in range(8):
                nc.sync.dma_start(out=x[k][:], in_=x0[k])

            # stage weight shards + first AG; staging copies on scalar q
            nc.sync.dma_start(out=qkv_st[0], in_=qkv_sh[0])
            ag(qkv_st[0], qkv_g[0][:])
            nc.scalar.dma_start(out=wo_st[0], in_=wo_sh[0])
            nc.scalar.dma_start(out=ffn_st[0], in_=ffn_sh[0])
            nc.scalar.dma_start(out=qkv_st[1], in_=qkv_sh[1])
            nc.scalar.dma_start(out=wo_st[1], in_=wo_sh[1])
            nc.scalar.dma_start(out=ffn_st[1], in_=ffn_sh[1])

            t_rmat = const.tile([128, 128], BF, tag="rmat")
            nc.sync.dma_start(out=t_rmat[:], in_=rmat[:])
            t_cos = const.tile([128, OWN], F32, tag="cos")
            nc.sync.dma_start(out=t_cos[:], in_=cosT[:])
            t_sin = const.tile([128, OWN], F32, tag="sin")
            nc.sync.dma_start(out=t_sin[:], in_=sinT[:])
            t_masks = [const.tile([128, 512], BF, tag=f"mask{m}",
                                  name=f"mask{m}") for m in range(16)]
            for m in range(16):
                nc.scalar.dma_start(out=t_masks[m][:], in_=masks[m])

            def ssq_acc(k, ssq):
                """Accumulate sum(x[k]^2) into ssq (psum row)."""
                x2 = sb.tile([128, OWN], BF, tag="x2")
                nc.scalar.activation(out=x2[:], in_=x[k][:], func=Square)
                nc.tensor.matmul(out=ssq[:], lhsT=ones_col[:], rhs=x2[:],
                                 start=(k == 0), stop=(k == 7))

            def rmsnorm(ssq=None):
                """x -> n bf16 chunks (npool tags n0..n7, reused per call)."""
                if ssq is None:
                    ssq = psB.tile([1, OWN], F32, tag="psB")
                    for k in range(8):
                        ssq_acc(k, ssq)
                rms = sbh.tile([1, OWN], F32, tag="rms", bufs=1)
                nc.scalar.activation(out=rms[:], in_=ssq[:], func=Sqrt,
                                     scale=1.0 / D, bias=eps_t[:])
                inv = sbh.tile([1, OWN], F32, tag="inv", bufs=1)
                nc.vector.reciprocal(out=inv[:], in_=rms[:])
                inv_bf = sbh.tile([1, OWN], BF, tag="invbf")
                nc.vector.tensor_copy(out=inv_bf[:], in_=inv[:])
                binv = psB.tile([128, OWN], F32, tag="psB")
                nc.tensor.matmul(out=binv[:], lhsT=ones_row[:], rhs=inv_bf[:],
                                 start=True, stop=True)
                n = [npool.tile([128, OWN], BF, tag=f"n{k}", name=f"n{k}")
                     for k in range(8)]
                for k in range(8):
                    nc.vector.tensor_tensor(out=n[k][:], in0=x[k][:],
                                            in1=binv[:], op=MULT)
                return n

            def rope(pm, y):
                """pm: psum [128, OWN] pre-rope -> bf16 tile y with rope."""
                yr = sb.tile([128, OWN], BF, tag="prerope")
                nc.vector.tensor_copy(out=yr[:], in_=pm[:])
                rot = psA.tile([128, OWN], F32, tag="psA")
                nc.tensor.matmul(out=rot[:], lhsT=t_rmat[:], rhs=yr[:],
                                 start=True, stop=True)
                tmp1 = sb.tile([128, OWN], F32, tag="ropet1", bufs=1)
                nc.vector.tensor_tensor(out=tmp1[:], in0=yr[:], in1=t_cos[:],
                                        op=MULT)
                tmp2 = sb.tile([128, OWN], F32, tag="ropet2", bufs=1)
                nc.vector.tensor_tensor(out=tmp2[:], in0=rot[:], in1=t_sin[:],
                                        op=MULT)
                nc.vector.tensor_tensor(out=y[:], in0=tmp1[:], in1=tmp2[:],
                                        op=ADD)
                return y

            nssq = [None]
            for l in range(L):
                n = rmsnorm(ssq=nssq[0])
                nssq[0] = None
                # ---- k/v first so the kv AllGather starts early ----
                wkv = []
                for k in range(8):
                    wt = wbig.tile([128, 512], BF, tag=f"wbig{k}",
                                   name=f"wkv{k}")
                    nc.sync.dma_start(out=wt[:], in_=qkv_g[l][k, :, 0:512])
                    wkv.append(wt)
                kr = [sb.tile([128, OWN], BF, tag=f"kr{mo}", name=f"kr{mo}")
                      for mo in range(2)]
                for mo in range(2):
                    pm = psM.tile([128, OWN], F32, tag="pmm")
                    for k in range(8):
                        nc.tensor.matmul(out=pm[:],
                                         lhsT=wkv[k][:, 128 * mo:128 * (mo + 1)],
                                         rhs=n[k][:], start=(k == 0),
                                         stop=(k == 7))
                    rope(pm, kr[mo])
                # v natural [own tok, 256]; lhsT = n col-slices
                for t in range(4):
                    pv = psM.tile([128, 256], F32, tag="pmm")
                    for k in range(8):
                        nc.tensor.matmul(out=pv[:],
                                         lhsT=n[k][:, 128 * t:128 * (t + 1)],
                                         rhs=wkv[k][:, 256:512],
                                         start=(k == 0), stop=(k == 7))
                    vt = sb.tile([128, 256], BF, tag="vnat")
                    nc.vector.tensor_copy(out=vt[:], in_=pv[:])
                    nc.sync.dma_start(
                        out=kvs[l][t // 2, 1, 128 * (t % 2):128 * (t % 2 + 1), :],
                        in_=vt[:])
                for b in range(B):
                    for mo in range(2):
                        nc.sync.dma_start(
                            out=kvs[l][b, 0, 128 * mo:128 * (mo + 1), :],
                            in_=kr[mo][:, 256 * b:256 * (b + 1)])
                ag(kvs[l][:], kvr[l][:])

                # ---- q (overlaps the kv AllGather) ----
                wqt = []
                for k in range(8):
                    wt = wbig.tile([128, 1024], BF, tag=f"wbig{k}",
                                   name=f"wq{k}")
                    nc.sync.dma_start(out=wt[:], in_=qkv_g[l][k, :, 512:1536])
                    wqt.append(wt)
                qr = [qpool.tile([128, OWN], BF, tag=f"qr{mo}", name=f"qr{mo}")
                      for mo in range(8)]
                for mo in range(8):
                    pm = psM.tile([128, OWN], F32, tag="pmm")
                    for k in range(8):
                        nc.tensor.matmul(out=pm[:],
                                         lhsT=wqt[k][:, 128 * mo:128 * (mo + 1)],
                                         rhs=n[k][:], start=(k == 0),
                                         stop=(k == 7))
                    rope(pm, qr[mo])

                # queue this layer's wo + ffn AGs behind the kv AG
                ag(wo_st[l], wo_g[l][:])
                ag(ffn_st[l], ffn_g[l][:])
                if l == 0:
                    ag(qkv_st[1], qkv_g[1][:])

                # ---- attention (per batch: assemble k/v, run units) ----
                casm = [apool.tile([128, OWN], BF, tag=f"casm{k}",
                                   name=f"casm{k}") for k in range(8)]
                for b in range(B):
                    kT = [apool.tile([64, S], BF, tag=f"kt{g}", name=f"kt{g}")
                          for g in range(KVH)]
                    for g in range(KVH):
                        src = kvr[l][:, b, 0, 64 * g:64 * (g + 1), :]
                        nc.sync.dma_start(
                            out=kT[g][:, 0:1024].rearrange(
                                "p (r c) -> p r c", r=C),
                            in_=src[:, :, 0:128].transpose([1, 0, 2]))
                        for r in range(C):
                            nc.sync.dma_start(
                                out=kT[g][:, 128 * (NB - 1 - r):128 * (NB - r)],
                                in_=src[r, :, 128:256])
                    v4 = [apool.tile([128, 260], BF, tag=f"v4{j}",
                                     name=f"v4{j}") for j in range(NB)]
                    for j in range(NB):
                        r, i = (j, 0) if j < C else (NB - 1 - j, 1)
                        dst = v4[j][:].rearrange("p (g c) -> p g c", g=4)
                        nc.sync.dma_start(
                            out=dst[:, :, 0:64],
                            in_=kvr[l][r, b, 1, 128 * i:128 * (i + 1), :]
                                .rearrange("p (g c) -> p g c", g=4))
                        nc.any.memset(dst[:, :, 64:65], 1.0)

                    for g in range(KVH):
                        for t in range(2):      # t=0: low block, t=1: high
                            qp = sb.tile([64, 512], BF, tag="qpack")
                            for i in range(4):
                                h = 4 * g + i
                                mo, ro = divmod(h, 2)
                                nc.vector.tensor_copy(
                                    out=qp[:, 128 * i:128 * (i + 1)],
                                    in_=qr[mo][64 * ro:64 * (ro + 1),
                                               _colseg(b, t)])
                            ctx = psB.tile([65, 512], F32, tag="psB")
                            nj = 8 if t == 0 else 16
                            for j in range(nj):
                                sc = psA.tile([128, 512], F32, tag="psA")
                                nc.tensor.matmul(
                                    out=sc[:],
                                    lhsT=kT[g][:, 128 * j:128 * (j + 1)],
                                    rhs=qp[:], start=True, stop=True)
                                ex = sb.tile([128, 512], BF, tag="exp")
                                nc.scalar.activation(out=ex[:], in_=sc[:],
                                                     func=Exp)
                                if t == 0 or j >= 8:
                                    exm = sb.tile([128, 512], BF, tag="expm")
                                    m = t_masks[j if t == 0 else j]
                                    nc.vector.tensor_tensor(
                                        out=exm[:], in0=ex[:], in1=m[:],
                                        op=MULT)
                                    ex = exm
                                nc.tensor.matmul(
                                    out=ctx[:],
                                    lhsT=v4[j][:, 65 * g:65 * (g + 1)],
                                    rhs=ex[:], start=(j == 0),
                                    stop=(j == nj - 1))
                            rec = sb.tile([1, 512], F32, tag="rec")
                            nc.vector.reciprocal(out=rec[:], in_=ctx[64:65, :])
                            rec_bf = sb.tile([1, 512], BF, tag="recbf")
                            nc.vector.tensor_copy(out=rec_bf[:], in_=rec[:])
                            brec = psA.tile([64, 512], F32, tag="psA")
                            nc.tensor.matmul(out=brec[:],
                                             lhsT=ones_row[:1, 0:64],
                                             rhs=rec_bf[:], start=True,
                                             stop=True)
                            brec_s = sb.tile([64, 512], BF, tag="brecs")
                            nc.vector.tensor_copy(out=brec_s[:], in_=brec[:])
                            for i in range(4):
                                h = 4 * g + i
                                mo, ro = divmod(h, 2)
                                nc.vector.tensor_tensor(
                                    out=casm[mo][64 * ro:64 * (ro + 1),
                                                 _colseg(b, t)],
                                    in0=ctx[0:64, 128 * i:128 * (i + 1)],
                                    in1=brec_s[:, 128 * i:128 * (i + 1)],
                                    op=MULT)

                # ---- wo + residual ----
                wot = []
                for k in range(8):
                    wt = wbig.tile([128, 1024], BF, tag=f"wbig{k}",
                                   name=f"wo{k}")
                    nc.sync.dma_start(out=wt[:], in_=wo_g[l][k])
                    wot.append(wt)
                ssq2 = psB.tile([1, OWN], F32, tag="psB")
                for mo in range(8):
                    pm = psM.tile([128, OWN], F32, tag="pmm")
                    for k in range(8):
                        nc.tensor.matmul(out=pm[:],
                                         lhsT=wot[k][:, 128 * mo:128 * (mo + 1)],
                                         rhs=casm[k][:], start=(k == 0),
                                         stop=(k == 7))
                    nc.vector.tensor_tensor(out=x[mo][:], in0=x[mo][:],
                                            in1=pm[:], op=ADD)
                    ssq_acc(mo, ssq2)

                # ---- FFN ----
                n2 = rmsnorm(ssq=ssq2)
                ht = [hpool.tile([128, OWN], BF, tag=f"h{mo}", name=f"h{mo}")
                      for mo in range(32)]
                for mb in range(8):
                    wgu = []
                    for k in range(8):
                        a = wbig.tile([128, 1024], BF, tag=f"wbig{k}",
                                      name=f"wgu{k}")
                        nc.sync.dma_start(
                            out=a[:],
                            in_=ffn_g[l][k, :, 1024 * mb:1024 * (mb + 1)])
                        wgu.append(a)
                    for ms in range(4):
                        mo = 4 * mb + ms
                        pg = psM.tile([128, OWN], F32, tag="pmm")
                        for k in range(8):
                            nc.tensor.matmul(
                                out=pg[:],
                                lhsT=wgu[k][:, 128 * ms:128 * (ms + 1)],
                                rhs=n2[k][:], start=(k == 0), stop=(k == 7))
                        gs = sb.tile([128, OWN], BF, tag="gsilu")
                        nc.scalar.activation(out=gs[:], in_=pg[:], func=Silu)
                        pu = psM.tile([128, OWN], F32, tag="pmm")
                        for k in range(8):
                            nc.tensor.matmul(
                                out=pu[:],
                                lhsT=wgu[k][:, 512 + 128 * ms:512 + 128 * (ms + 1)],
                                rhs=n2[k][:], start=(k == 0), stop=(k == 7))
                        nc.vector.tensor_tensor(out=ht[mo][:], in0=pu[:],
                                                in1=gs[:], op=MULT)
                # down-proj: two output chunks per pass, stream wd tiles
                ssq3 = psB.tile([1, OWN], F32, tag="psB")
                for mp in range(4):
                    pd0 = psM.tile([128, OWN], F32, tag="pmm")
                    pd1 = psM.tile([128, OWN], F32, tag="pmm")
                    for kk in range(32):
                        c_, j = divmod(kk, 4)
                        wt = wbig.tile([128, 256], BF, tag="wsm", bufs=4,
                                       name="wdt")
                        eng = nc.sync if kk % 2 == 0 else nc.scalar
                        eng.dma_start(
                            out=wt[:],
                            in_=ffn_g[l][c_, :, 8192 + 1024 * j + 256 * mp:
                                         8192 + 1024 * j + 256 * (mp + 1)])
                        nc.tensor.matmul(out=pd0[:], lhsT=wt[:, 0:128],
                                         rhs=ht[kk][:], start=(kk == 0),
                                         stop=(kk == 31))
                        nc.tensor.matmul(out=pd1[:], lhsT=wt[:, 128:256],
                                         rhs=ht[kk][:], start=(kk == 0),
                                         stop=(kk == 31))
                    nc.vector.tensor_tensor(out=x[2 * mp][:], in0=x[2 * mp][:],
                                            in1=pd0[:], op=ADD)
                    nc.vector.tensor_tensor(out=x[2 * mp + 1][:],
                                            in0=x[2 * mp + 1][:],
                                            in1=pd1[:], op=ADD)
                    ssq_acc(2 * mp, ssq3)
                    ssq_acc(2 * mp + 1, ssq3)
                nssq[0] = ssq3

            # ---- final norm -> nf (npool, survives body pools) ----
            nf = rmsnorm(ssq=nssq[0])
            for k in range(8):
                nc.sync.dma_start(out=nf_st[:, 512 * k:512 * (k + 1)],
                                  in_=nf[k][:])
            ag(nf_st[:], nf_g[:])

        # ---- vocab-sharded head: all tokens x our V/8 slice ----
        with ExitStack() as hd:
            hp = hd.enter_context(tc.tile_pool(name="hp", bufs=1))
            hw = hd.enter_context(tc.tile_pool(name="hw", bufs=2))
            whead = []
            for k in range(8):
                wt = hp.tile([128, VS], BF, tag=f"wh{k}", name=f"wh{k}")
                nc.sync.dma_start(out=wt[:], in_=woutc[128 * k:128 * (k + 1), :])
                whead.append(wt)
            for cp in range(C):
                nfo = hw.tile([128, 8 * OWN], BF, tag="nfo")
                nc.sync.dma_start(out=nfo[:], in_=nf_g[cp])
                for tb in range(4):
                    for vt in range(NVT):
                        vw = min(512, VS - 512 * vt)
                        ph = psM.tile([128, 512], F32, tag="pmm")
                        for k in range(8):
                            nc.tensor.matmul(
                                out=ph[:, :vw],
                                lhsT=nfo[:, 512 * k + 128 * tb:
                                         512 * k + 128 * (tb + 1)],
                                rhs=whead[k][:, 512 * vt:512 * vt + vw],
                                start=(k == 0), stop=(k == 7))
                        ot = hw.tile([128, 512], F16, tag="hout")
                        if vt % 2 == 0:
                            nc.vector.tensor_copy(out=ot[:, :vw],
                                                  in_=ph[:, :vw])
                        else:
                            nc.scalar.activation(out=ot[:, :vw],
                                                 in_=ph[:, :vw], func=Copy)
                        nc.sync.dma_start(
                            out=logits[512 * cp + 128 * tb:
                                       512 * cp + 128 * (tb + 1),
                                       512 * vt:512 * vt + vw],
                            in_=ot[:, :vw])

    return nc


# ------------------------------------------------------------------ host ---

_NC_CACHE = {}


def _get_nc():
    if "nc" not in _NC_CACHE:
        nc = build_nc()
        nc.finalize()
        _NC_CACHE["nc"] = nc
    return _NC_CACHE["nc"]


def _host_prep(inputs):
    inv_freq = 1.0 / ROPE_BASE ** (np.arange(0, HD, 2, dtype=np.float32) / HD)
    t = np.arange(S, dtype=np.float32)
    freqs = t[:, None] * inv_freq[None, :]
    ang = np.concatenate([freqs, freqs], axis=-1)       # [S, 64]
    cos_full, sin_full = np.cos(ang), np.sin(ang)
    cosT2 = np.empty((128, S), np.float32)
    sinT2 = np.empty((128, S), np.float32)
    for p in range(128):
        d = p % 64
        cosT2[p] = cos_full[:, d]
        sinT2[p] = sin_full[:, d] * (-1.0 if d < 32 else 1.0)

    R = np.zeros((128, 128), np.float32)
    for blk in range(2):
        o = blk * 64
        for j in range(32):
            R[o + 32 + j, o + j] = 1.0
            R[o + j, o + 32 + j] = 1.0

    naw = np.asarray(inputs["norm_attn_w"], np.float32)
    nfw = np.asarray(inputs["norm_ff_w"], np.float32)
    emb = np.asarray(inputs["token_emb"], np.float32)
    wq_ = (np.asarray(inputs["wq"], np.float32) * naw[:, :, None] * SCALE
           ).astype(BF16)
    wk_ = (np.asarray(inputs["wk"], np.float32) * naw[:, :, None]).astype(BF16)
    wv_ = (np.asarray(inputs["wv"], np.float32) * naw[:, :, None]).astype(BF16)
    wo_ = np.asarray(inputs["wo"], np.float32).astype(BF16)
    wg_ = (np.asarray(inputs["w_gate"], np.float32) * nfw[:, :, None]
           ).astype(BF16)
    wu_ = (np.asarray(inputs["w_up"], np.float32) * nfw[:, :, None]
           ).astype(BF16)
    wd_ = np.asarray(inputs["w_down"], np.float32).astype(BF16)
    wout_ = (np.asarray(inputs["w_out"], np.float32)
             * np.asarray(inputs["norm_final_w"], np.float32)[:, None]
             ).astype(BF16)
    rmat_b = np.ascontiguousarray(R.astype(BF16))

    idx_full = np.asarray(inputs["in_idx"]).astype(np.int64)
    tri = (np.arange(128)[:, None] <= np.arange(128)[None, :]).astype(np.float32)
    tri4 = np.tile(tri, (1, 4))
    in_maps = []
    for c in range(C):
        blks = own_blocks(c)
        rs = slice(128 * c, 128 * (c + 1))
        # own-token ids in column order (b, tt): (0,b0),(0,b1),(1,b0),(1,b1)
        ids = np.concatenate([idx_full[b, bl * BS:(bl + 1) * BS]
                              for b in range(B) for bl in blks])
        x0 = np.ascontiguousarray(
            emb[ids].T.reshape(8, 128, OWN).astype(np.float32))
        pos = np.concatenate([np.arange(bl * BS, (bl + 1) * BS) for bl in blks])
        cosT = np.ascontiguousarray(
            np.concatenate([cosT2[:, pos], cosT2[:, pos]], axis=1))
        sinT = np.ascontiguousarray(
            np.concatenate([sinT2[:, pos], sinT2[:, pos]], axis=1))
        mk = np.zeros((16, 128, 512), np.float32)
        for t_, blk in enumerate(blks):
            for jj in range(8):
                j = jj if t_ == 0 else jj + 8
                if j < blk:
                    mk[8 * t_ + jj] = 1.0
                elif j == blk:
                    mk[8 * t_ + jj] = tri4
        qkv = np.ascontiguousarray(np.concatenate(
            [wk_[:, rs, :], wv_[:, rs, :], wq_[:, rs, :]], axis=2))
        wosh = np.ascontiguousarray(wo_[:, rs, :])
        ffn = np.empty((L, 128, 12288), BF16)
        for mb in range(8):
            ffn[:, :, 1024 * mb:1024 * mb + 512] = \
                wg_[:, rs, 512 * mb:512 * (mb + 1)]
            ffn[:, :, 1024 * mb + 512:1024 * (mb + 1)] = \
                wu_[:, rs, 512 * mb:512 * (mb + 1)]
        ffn[:, :, 8192:] = (
            wd_[:, 512 * c:512 * (c + 1), :]
            .reshape(L, 4, 128, D).transpose(0, 2, 1, 3).reshape(L, 128, 4096))
        in_maps.append({
            "x0": x0,
            "cosT": cosT,
            "sinT": sinT,
            "rmat": rmat_b,
            "masks": np.ascontiguousarray(mk.astype(BF16)),
            "qkv_sh": qkv,
            "wo_sh": wosh,
            "ffn_sh": np.ascontiguousarray(ffn),
            "woutc": np.ascontiguousarray(wout_[:, VS * c:VS * (c + 1)]),
        })
    return in_maps


def _assemble(results):
    out = np.empty((B, S, V), np.float32)
    for c in range(C):          # vocab-shard owner
        lg = np.asarray(results[c]["logits"]).astype(np.float32)
        for cp in range(C):     # token owner
            blks = own_blocks(cp)
            for b in range(B):
                for tt in range(2):
                    r0 = cp * 512 + 128 * (2 * b + tt)
                    out[b, blks[tt] * BS:(blks[tt] + 1) * BS,
                        VS * c:VS * (c + 1)] = lg[r0:r0 + 128]
    return out


def run(inputs, trace=False, trace_cores=None):
    nc = _get_nc()
    in_maps = _host_prep(inputs)
    res = run_bass_kernel_spmd(nc, in_maps, list(range(C)), trace=trace,
                               trace_cores=trace_cores)
    return _assemble(res.results), res


def kernel(**inputs):
    out, _ = run(inputs)
    return out


# revision 46
# speedup vs baseline: 1.0099x; 1.0099x over previous
"""Self-contained Trainium2 Bass kernel for the 2-layer Llama3 model.

Sharding: token-sharded compute over 8 cores (core c owns token blocks
{c, 15-c} of each batch, 512 tokens/core), with *minimal host->device
input bytes*: every weight is shipped exactly once, row-sharded by its
contraction dim (1/8 per core), and AllGathered on-device over the
intra-chip links, overlapped with compute. Embedding rows are gathered
on host (2MB/core instead of the 131MB fp32 table). The vocab head is
vocab-sharded: each core keeps its [D, V/8] slice of w_out, final
hidden states are AllGathered (8MB), and each core computes logits for
ALL tokens x its vocab slice, emitted as fp16 (host casts to f32).

Per-core input ~20MB (vs ~257MB replicated), output 32MB fp16.

Device layouts: activations transposed [feature, token] in 128-part
chunks; scores computed transposed [sk, sq]; softmax denominator via a
ones-augmented column on v; RoPE via a +-1 rotation matrix on the PE.
SPMD-uniform program: per-core causality lives in mask *data*.
"""
from contextlib import ExitStack

import numpy as np
import ml_dtypes

import concourse.bass as bass
from concourse.bacc import Bacc
import concourse.mybir as mybir
import concourse.tile as tile
from concourse.bass_utils import run_bass_kernel_spmd

BF16 = ml_dtypes.bfloat16
F32 = mybir.dt.float32
F16 = mybir.dt.float16
BF = mybir.dt.bfloat16

V, D, H, KVH, HD, F, L, B, S = 32000, 1024, 16, 4, 64, 4096, 2, 2, 2048
ROPE_BASE = 500000.0
EPS = 1e-5
SCALE = 1.0 / HD ** 0.5
C = 8            # cores
NB = 16          # 128-token blocks per batch
BS = 128         # block size
OWN = 512        # own tokens per core (2 blocks x 2 batches)
VS = V // C      # vocab shard per core (4000)
NVT = (VS + 511) // 512   # 8 vocab tiles (last is 416 wide)

Exp = mybir.ActivationFunctionType.Exp
Silu = mybir.ActivationFunctionType.Silu
Sqrt = mybir.ActivationFunctionType.Sqrt
Copy = mybir.ActivationFunctionType.Copy
Square = mybir.ActivationFunctionType.Square
MULT = mybir.AluOpType.mult
ADD = mybir.AluOpType.add


def own_blocks(c):
    return [c, NB - 1 - c]


def _colseg(b, t):
    """Own-token column range for (batch b, tile t in {0=low,1=high})."""
    return slice(256 * b + 128 * t, 256 * b + 128 * (t + 1))


# ---------------------------------------------------------------- device ---

def build_nc():
    nc = Bacc()

    x0 = nc.dram_tensor("x0", [8, 128, OWN], F32, kind="ExternalInput")
    cosT = nc.dram_tensor("cosT", [128, OWN], F32, kind="ExternalInput")
    sinT = nc.dram_tensor("sinT", [128, OWN], F32, kind="ExternalInput")
    rmat = nc.dram_tensor("rmat", [128, 128], BF, kind="ExternalInput")
    masks = nc.dram_tensor("masks", [16, 128, 512], BF, kind="ExternalInput")
    # weight shards, row-sharded by contraction dim (1/8 per core):
    # qkv_sh cols: wk 0:256 | wv 256:512 | wq 512:1536
    qkv_sh = nc.dram_tensor("qkv_sh", [L, 128, 1536], BF, kind="ExternalInput")
    wo_sh = nc.dram_tensor("wo_sh", [L, 128, 1024], BF, kind="ExternalInput")
    # ffn_sh cols: 8 groups of [wg 512 | wu 512] (0:8192), then wd
    # as 4 groups of 1024 (F-chunks 4c..4c+3, each [128, D]) at 8192:12288
    ffn_sh = nc.dram_tensor("ffn_sh", [L, 128, 12288], BF, kind="ExternalInput")
    woutc = nc.dram_tensor("woutc", [D, VS], BF, kind="ExternalInput")
    logits = nc.dram_tensor("logits", [C * OWN, VS], F16, kind="ExternalOutput")

    # collective staging (internal) and gathered (Shared) buffers
    qkv_st = nc.dram_tensor("qkv_st", [L, 128, 1536], BF)
    wo_st = nc.dram_tensor("wo_st", [L, 128, 1024], BF)
    ffn_st = nc.dram_tensor("ffn_st", [L, 128, 12288], BF)
    nf_st = nc.dram_tensor("nf_st", [128, 8 * OWN], BF)
    qkv_g = [nc.dram_tensor(f"qkvg{l}", [C, 128, 1536], BF,
                            addr_space="Shared") for l in range(L)]
    wo_g = [nc.dram_tensor(f"wog{l}", [C, 128, 1024], BF,
                           addr_space="Shared") for l in range(L)]
    ffn_g = [nc.dram_tensor(f"ffng{l}", [C, 128, 12288], BF,
                            addr_space="Shared") for l in range(L)]
    nf_g = nc.dram_tensor("nfg", [C, 128, 8 * OWN], BF, addr_space="Shared")
    kvs = [nc.dram_tensor(f"kvs{l}", [B, 2, 256, 256], BF) for l in range(L)]
    kvr = [nc.dram_tensor(f"kvr{l}", [C, B, 2, 256, 256], BF,
                          addr_space="Shared") for l in range(L)]

    def ag(src_ap, dst_ap):
        nc.gpsimd.collective_compute(
            "AllGather", mybir.AluOpType.bypass,
            replica_groups=[list(range(C))],
            ins=[src_ap], outs=[dst_ap])

    with tile.TileContext(nc) as tc, ExitStack() as st:
        npool = st.enter_context(tc.tile_pool(name="npool", bufs=1))
        sbh = st.enter_context(tc.tile_pool(name="sbh", bufs=2))
        psA = st.enter_context(tc.tile_pool(name="psA", bufs=2, space="PSUM"))
        psB = st.enter_context(tc.tile_pool(name="psB", bufs=2, space="PSUM"))
        psM = st.enter_context(tc.tile_pool(name="psM", bufs=3, space="PSUM"))

        with ExitStack() as body:
            const = body.enter_context(tc.tile_pool(name="const", bufs=1))
            resid = body.enter_context(tc.tile_pool(name="resid", bufs=1))
            qpool = body.enter_context(tc.tile_pool(name="qpool", bufs=1))
            apool = body.enter_context(tc.tile_pool(name="apool", bufs=1))
            hpool = body.enter_context(tc.tile_pool(name="hpool", bufs=1))
            sb = body.enter_context(tc.tile_pool(name="sb", bufs=2))
            wbig = body.enter_context(tc.tile_pool(name="wbig", bufs=2))

            # constants FIRST: memsets live on the gpsimd queue, which the
            # collective waits also occupy — emit before any ag()
            ones_col = const.tile([128, 1], BF, tag="ones_col")
            nc.any.memset(ones_col[:], 1.0)
            ones_row = const.tile([1, 128], BF, tag="ones_row")
            nc.any.memset(ones_row[:], 1.0)
            eps_t = const.tile([1, 1], F32, tag="eps")
            nc.any.memset(eps_t[:], EPS)

            # residual stream xT: 8 chunks [128, OWN] f32, resident
            x = [resid.tile([128, OWN], F32, tag=f"x{k}", name=f"x{k}")
                 for k in range(8)]
            for k in range(8):
                nc.sync.dma_start(out=x[k][:], in_=x0[k])

            # stage layer-0 qkv shard early on the lightly-loaded sync ring
            nc.sync.dma_start(out=qkv_st[0], in_=qkv_sh[0])

            t_rmat = const.tile([128, 128], BF, tag="rmat")
            nc.sync.dma_start(out=t_rmat[:], in_=rmat[:])
            t_cos = const.tile([128, OWN], F32, tag="cos")
            nc.sync.dma_start(out=t_cos[:], in_=cosT[:])
            t_sin = const.tile([128, OWN], F32, tag="sin")
            nc.sync.dma_start(out=t_sin[:], in_=sinT[:])

            def ssq_acc(k, ssq):
                """Accumulate sum(x[k]^2) into ssq (psum row)."""
                x2 = sb.tile([128, OWN], BF, tag="x2")
                nc.scalar.activation(out=x2[:], in_=x[k][:], func=Square)
                nc.tensor.matmul(out=ssq[:], lhsT=ones_col[:], rhs=x2[:],
                                 start=(k == 0), stop=(k == 7))

            def rmsnorm(ssq=None):
                """x -> n bf16 chunks (npool tags n0..n7, reused per call)."""
                if ssq is None:
                    ssq = psB.tile([1, OWN], F32, tag="psB")
                    for k in range(8):
                        ssq_acc(k, ssq)
                rms = sbh.tile([1, OWN], F32, tag="rms", bufs=1)
                nc.scalar.activation(out=rms[:], in_=ssq[:], func=Sqrt,
                                     scale=1.0 / D, bias=eps_t[:])
                inv = sbh.tile([1, OWN], F32, tag="inv", bufs=1)
                nc.vector.reciprocal(out=inv[:], in_=rms[:])
                inv_bf = sbh.tile([1, OWN], BF, tag="invbf")
                nc.vector.tensor_copy(out=inv_bf[:], in_=inv[:])
                binv = psB.tile([128, OWN], F32, tag="psB")
                nc.tensor.matmul(out=binv[:], lhsT=ones_row[:], rhs=inv_bf[:],
                                 start=True, stop=True)
                n = [npool.tile([128, OWN], BF, tag=f"n{k}", name=f"n{k}")
                     for k in range(8)]
                for k in range(8):
                    nc.vector.tensor_tensor(out=n[k][:], in0=x[k][:],
                                            in1=binv[:], op=MULT)
                return n

            def rope(pm, y):
                """pm: psum [128, OWN] pre-rope -> bf16 tile y with rope."""
                yr = sb.tile([128, OWN], BF, tag="prerope")
                nc.vector.tensor_copy(out=yr[:], in_=pm[:])
                rot = psA.tile([128, OWN], F32, tag="psA")
                nc.tensor.matmul(out=rot[:], lhsT=t_rmat[:], rhs=yr[:],
                                 start=True, stop=True)
                tmp1 = sb.tile([128, OWN], F32, tag="ropet1", bufs=1)
                nc.vector.tensor_tensor(out=tmp1[:], in0=yr[:], in1=t_cos[:],
                                        op=MULT)
                tmp2 = sb.tile([128, OWN], F32, tag="ropet2", bufs=1)
                nc.vector.tensor_tensor(out=tmp2[:], in0=rot[:], in1=t_sin[:],
                                        op=MULT)
                nc.vector.tensor_tensor(out=y[:], in0=tmp1[:], in1=tmp2[:],
                                        op=ADD)
                return y

            # L0 rmsnorm FIRST so its ACT squares lead the scalar queue,
            # then the first AG, then the remaining staging + mask loads.
            n0 = rmsnorm()
            ag(qkv_st[0], qkv_g[0][:])
            nc.scalar.dma_start(out=wo_st[0], in_=wo_sh[0])
            nc.scalar.dma_start(out=ffn_st[0], in_=ffn_sh[0])
            nc.sync.dma_start(out=qkv_st[1], in_=qkv_sh[1])
            nc.scalar.dma_start(out=wo_st[1], in_=wo_sh[1])
            nc.scalar.dma_start(out=ffn_st[1], in_=ffn_sh[1])
            t_masks = [const.tile([128, 512], BF, tag=f"mask{m}",
                                  name=f"mask{m}") for m in range(16)]
            for m in range(16):
                nc.scalar.dma_start(out=t_masks[m][:], in_=masks[m])

            nssq = [None]
            for l in range(L):
                n = n0 if l == 0 else rmsnorm(ssq=nssq[0])
                nssq[0] = None
                # ---- k/v first so the kv AllGather starts early ----
                wkv = []
                for k in range(8):
                    wt = wbig.tile([128, 512], BF, tag=f"wbig{k}",
                                   name=f"wkv{k}")
                    nc.sync.dma_start(out=wt[:], in_=qkv_g[l][k, :, 0:512])
                    wkv.append(wt)
                kr = [sb.tile([128, OWN], BF, tag=f"kr{mo}", name=f"kr{mo}")
                      for mo in range(2)]
                for mo in range(2):
                    pm = psM.tile([128, OWN], F32, tag="pmm")
                    for k in range(8):
                        nc.tensor.matmul(out=pm[:],
                                         lhsT=wkv[k][:, 128 * mo:128 * (mo + 1)],
                                         rhs=n[k][:], start=(k == 0),
                                         stop=(k == 7))
                    rope(pm, kr[mo])
                # v natural [own tok, 256]; lhsT = n col-slices
                for t in range(4):
                    pv = psM.tile([128, 256], F32, tag="pmm")
                    for k in range(8):
                        nc.tensor.matmul(out=pv[:],
                                         lhsT=n[k][:, 128 * t:128 * (t + 1)],
                                         rhs=wkv[k][:, 256:512],
                                         start=(k == 0), stop=(k == 7))
                    vt = sb.tile([128, 256], BF, tag="vnat")
                    nc.vector.tensor_copy(out=vt[:], in_=pv[:])
                    nc.sync.dma_start(
                        out=kvs[l][t // 2, 1, 128 * (t % 2):128 * (t % 2 + 1), :],
                        in_=vt[:])
                for b in range(B):
                    for mo in range(2):
                        nc.sync.dma_start(
                            out=kvs[l][b, 0, 128 * mo:128 * (mo + 1), :],
                            in_=kr[mo][:, 256 * b:256 * (b + 1)])
                ag(kvs[l][:], kvr[l][:])

                # ---- q (overlaps the kv AllGather) ----
                wqt = []
                for k in range(8):
                    wt = wbig.tile([128, 1024], BF, tag=f"wbig{k}",
                                   name=f"wq{k}")
                    nc.sync.dma_start(out=wt[:], in_=qkv_g[l][k, :, 512:1536])
                    wqt.append(wt)
                qr = [qpool.tile([128, OWN], BF, tag=f"qr{mo}", name=f"qr{mo}")
                      for mo in range(8)]
                for mo in range(8):
                    pm = psM.tile([128, OWN], F32, tag="pmm")
                    for k in range(8):
                        nc.tensor.matmul(out=pm[:],
                                         lhsT=wqt[k][:, 128 * mo:128 * (mo + 1)],
                                         rhs=n[k][:], start=(k == 0),
                                         stop=(k == 7))
                    rope(pm, qr[mo])

                # queue this layer's wo + ffn AGs behind the kv AG
                ag(wo_st[l], wo_g[l][:])
                ag(ffn_st[l], ffn_g[l][:])
                if l == 0:
                    ag(qkv_st[1], qkv_g[1][:])

                # ---- attention (per batch: assemble k/v, run units) ----
                casm = [apool.tile([128, OWN], BF, tag=f"casm{k}",
                                   name=f"casm{k}") for k in range(8)]
                for b in range(B):
                    kT = [apool.tile([64, S], BF, tag=f"kt{g}", name=f"kt{g}")
                          for g in range(KVH)]
                    for g in range(KVH):
                        src = kvr[l][:, b, 0, 64 * g:64 * (g + 1), :]
                        nc.sync.dma_start(
                            out=kT[g][:, 0:1024].rearrange(
                                "p (r c) -> p r c", r=C),
                            in_=src[:, :, 0:128].transpose([1, 0, 2]))
                        for r in range(C):
                            nc.sync.dma_start(
                                out=kT[g][:, 128 * (NB - 1 - r):128 * (NB - r)],
                                in_=src[r, :, 128:256])
                    v4 = [apool.tile([128, 260], BF, tag=f"v4{j}",
                                     name=f"v4{j}") for j in range(NB)]
                    for j in range(NB):
                        r, i = (j, 0) if j < C else (NB - 1 - j, 1)
                        dst = v4[j][:].rearrange("p (g c) -> p g c", g=4)
                        nc.sync.dma_start(
                            out=dst[:, :, 0:64],
                            in_=kvr[l][r, b, 1, 128 * i:128 * (i + 1), :]
                                .rearrange("p (g c) -> p g c", g=4))
                        nc.any.memset(dst[:, :, 64:65], 1.0)

                    for g in range(KVH):
                        for t in range(2):      # t=0: low block, t=1: high
                            qp = sb.tile([64, 512], BF, tag="qpack")
                            for i in range(4):
                                h = 4 * g + i
                                mo, ro = divmod(h, 2)
                                nc.vector.tensor_copy(
                                    out=qp[:, 128 * i:128 * (i + 1)],
                                    in_=qr[mo][64 * ro:64 * (ro + 1),
                                               _colseg(b, t)])
                            ctx = psB.tile([65, 512], F32, tag="psB")
                            nj = 8 if t == 0 else 16
                            for j in range(nj):
                                sc = psA.tile([128, 512], F32, tag="psA")
                                nc.tensor.matmul(
                                    out=sc[:],
                                    lhsT=kT[g][:, 128 * j:128 * (j + 1)],
                                    rhs=qp[:], start=True, stop=True)
                                ex = sb.tile([128, 512], BF, tag="exp")
                                nc.scalar.activation(out=ex[:], in_=sc[:],
                                                     func=Exp)
                                if t == 0 or j >= 8:
                                    exm = sb.tile([128, 512], BF, tag="expm")
                                    m = t_masks[j if t == 0 else j]
                                    nc.vector.tensor_tensor(
                                        out=exm[:], in0=ex[:], in1=m[:],
                                        op=MULT)
                                    ex = exm
                                nc.tensor.matmul(
                                    out=ctx[:],
                                    lhsT=v4[j][:, 65 * g:65 * (g + 1)],
                                    rhs=ex[:], start=(j == 0),
                                    stop=(j == nj - 1))
                            rec = sb.tile([1, 512], F32, tag="rec")
                            nc.vector.reciprocal(out=rec[:], in_=ctx[64:65, :])
                            rec_bf = sb.tile([1, 512], BF, tag="recbf")
                            nc.vector.tensor_copy(out=rec_bf[:], in_=rec[:])
                            brec = psA.tile([64, 512], F32, tag="psA")
                            nc.tensor.matmul(out=brec[:],
                                             lhsT=ones_row[:1, 0:64],
                                             rhs=rec_bf[:], start=True,
                                             stop=True)
                            brec_s = sb.tile([64, 512], BF, tag="brecs")
                            nc.vector.tensor_copy(out=brec_s[:], in_=brec[:])
                            for i in range(4):
                                h = 4 * g + i
                                mo, ro = divmod(h, 2)
                                nc.vector.tensor_tensor(
                                    out=casm[mo][64 * ro:64 * (ro + 1),
                                                 _colseg(b, t)],
                                    in0=ctx[0:64, 128 * i:128 * (i + 1)],
                                    in1=brec_s[:, 128 * i:128 * (i + 1)],
                                    op=MULT)

                # ---- wo + residual ----
                wot = []
                for k in range(8):
                    wt = wbig.tile([128, 1024], BF, tag=f"wbig{k}",
                                   name=f"wo{k}")
                    nc.sync.dma_start(out=wt[:], in_=wo_g[l][k])
                    wot.append(wt)
                ssq2 = psB.tile([1, OWN], F32, tag="psB")
                for mo in range(8):
                    pm = psM.tile([128, OWN], F32, tag="pmm")
                    for k in range(8):
                        nc.tensor.matmul(out=pm[:],
                                         lhsT=wot[k][:, 128 * mo:128 * (mo + 1)],
                                         rhs=casm[k][:], start=(k == 0),
                                         stop=(k == 7))
                    nc.vector.tensor_tensor(out=x[mo][:], in0=x[mo][:],
                                            in1=pm[:], op=ADD)
                    ssq_acc(mo, ssq2)

                # ---- FFN ----
                n2 = rmsnorm(ssq=ssq2)
                ht = [hpool.tile([128, OWN], BF, tag=f"h{mo}", name=f"h{mo}")
                      for mo in range(32)]
                for mb in range(8):
                    wgu = []
                    for k in range(8):
                        a = wbig.tile([128, 1024], BF, tag=f"wbig{k}",
                                      name=f"wgu{k}")
                        nc.sync.dma_start(
                            out=a[:],
                            in_=ffn_g[l][k, :, 1024 * mb:1024 * (mb + 1)])
                        wgu.append(a)
                    for ms in range(4):
                        mo = 4 * mb + ms
                        pg = psM.tile([128, OWN], F32, tag="pmm")
                        for k in range(8):
                            nc.tensor.matmul(
                                out=pg[:],
                                lhsT=wgu[k][:, 128 * ms:128 * (ms + 1)],
                                rhs=n2[k][:], start=(k == 0), stop=(k == 7))
                        gs = sb.tile([128, OWN], BF, tag="gsilu")
                        nc.scalar.activation(out=gs[:], in_=pg[:], func=Silu)
                        pu = psM.tile([128, OWN], F32, tag="pmm")
                        for k in range(8):
                            nc.tensor.matmul(
                                out=pu[:],
                                lhsT=wgu[k][:, 512 + 128 * ms:512 + 128 * (ms + 1)],
                                rhs=n2[k][:], start=(k == 0), stop=(k == 7))
                        nc.vector.tensor_tensor(out=ht[mo][:], in0=pu[:],
                                                in1=gs[:], op=MULT)
                # down-proj: two output chunks per pass, stream wd tiles
                ssq3 = psB.tile([1, OWN], F32, tag="psB")
                for mp in range(4):
                    pd0 = psM.tile([128, OWN], F32, tag="pmm")
                    pd1 = psM.tile([128, OWN], F32, tag="pmm")
                    for kk in range(32):
                        c_, j = divmod(kk, 4)
                        wt = wbig.tile([128, 256], BF, tag="wsm", bufs=4,
                                       name="wdt")
                        eng = nc.sync if kk % 2 == 0 else nc.scalar
                        eng.dma_start(
                            out=wt[:],
                            in_=ffn_g[l][c_, :, 8192 + 1024 * j + 256 * mp:
                                         8192 + 1024 * j + 256 * (mp + 1)])
                        nc.tensor.matmul(out=pd0[:], lhsT=wt[:, 0:128],
                                         rhs=ht[kk][:], start=(kk == 0),
                                         stop=(kk == 31))
                        nc.tensor.matmul(out=pd1[:], lhsT=wt[:, 128:256],
                                         rhs=ht[kk][:], start=(kk == 0),
                                         stop=(kk == 31))
                    nc.vector.tensor_tensor(out=x[2 * mp][:], in0=x[2 * mp][:],
                                            in1=pd0[:], op=ADD)
                    nc.vector.tensor_tensor(out=x[2 * mp + 1][:],
                                            in0=x[2 * mp + 1][:],
                                            in1=pd1[:], op=ADD)
                    ssq_acc(2 * mp, ssq3)
                    ssq_acc(2 * mp + 1, ssq3)
                nssq[0] = ssq3

            # ---- final norm -> nf (npool, survives body pools) ----
            nf = rmsnorm(ssq=nssq[0])
            for k in range(8):
                nc.sync.dma_start(out=nf_st[:, 512 * k:512 * (k + 1)],
                                  in_=nf[k][:])
            ag(nf_st[:], nf_g[:])

        # ---- vocab-sharded head: all tokens x our V/8 slice ----
        with ExitStack() as hd:
            hp = hd.enter_context(tc.tile_pool(name="hp", bufs=1))
            hw = hd.enter_context(tc.tile_pool(name="hw", bufs=2))
            whead = []
            for k in range(8):
                wt = hp.tile([128, VS], BF, tag=f"wh{k}", name=f"wh{k}")
                nc.sync.dma_start(out=wt[:], in_=woutc[128 * k:128 * (k + 1), :])
                whead.append(wt)
            for cp in range(C):
                nfo = hw.tile([128, 8 * OWN], BF, tag="nfo")
                nc.sync.dma_start(out=nfo[:], in_=nf_g[cp])
                for tb in range(4):
                    for vt in range(NVT):
                        vw = min(512, VS - 512 * vt)
                        ph = psM.tile([128, 512], F32, tag="pmm")
                        for k in range(8):
                            nc.tensor.matmul(
                                out=ph[:, :vw],
                                lhsT=nfo[:, 512 * k + 128 * tb:
                                         512 * k + 128 * (tb + 1)],
                                rhs=whead[k][:, 512 * vt:512 * vt + vw],
                                start=(k == 0), stop=(k == 7))
                        ot = hw.tile([128, 512], F16, tag="hout")
                        if vt % 2 == 0:
                            nc.vector.tensor_copy(out=ot[:, :vw],
                                                  in_=ph[:, :vw])
                        else:
                            nc.scalar.activation(out=ot[:, :vw],
                                                 in_=ph[:, :vw], func=Copy)
                        nc.sync.dma_start(
                            out=logits[512 * cp + 128 * tb:
                                       512 * cp + 128 * (tb + 1),
                                       512 * vt:512 * vt + vw],
                            in_=ot[:, :vw])

    return nc


# ------------------------------------------------------------------ host ---

_NC_CACHE = {}


def _get_nc():
    if "nc" not in _NC_CACHE:
        nc = build_nc()
        nc.finalize()
        _NC_CACHE["nc"] = nc
    return _NC_CACHE["nc"]


def _host_prep(inputs):
    inv_freq = 1.0 / ROPE_BASE ** (np.arange(0, HD, 2, dtype=np.float32) / HD)
    t = np.arange(S, dtype=np.float32)
    freqs = t[:, None] * inv_freq[None, :]
    ang = np.concatenate([freqs, freqs], axis=-1)       # [S, 64]
    cos_full, sin_full = np.cos(ang), np.sin(ang)
    cosT2 = np.empty((128, S), np.float32)
    sinT2 = np.empty((128, S), np.float32)
    for p in range(128):
        d = p % 64
        cosT2[p] = cos_full[:, d]
        sinT2[p] = sin_full[:, d] * (-1.0 if d < 32 else 1.0)

    R = np.zeros((128, 128), np.float32)
    for blk in range(2):
        o = blk * 64
        for j in range(32):
            R[o + 32 + j, o + j] = 1.0
            R[o + j, o + 32 + j] = 1.0

    naw = np.asarray(inputs["norm_attn_w"], np.float32)
    nfw = np.asarray(inputs["norm_ff_w"], np.float32)
    emb = np.asarray(inputs["token_emb"], np.float32)
    wq_ = (np.asarray(inputs["wq"], np.float32) * naw[:, :, None] * SCALE
           ).astype(BF16)
    wk_ = (np.asarray(inputs["wk"], np.float32) * naw[:, :, None]).astype(BF16)
    wv_ = (np.asarray(inputs["wv"], np.float32) * naw[:, :, None]).astype(BF16)
    wo_ = np.asarray(inputs["wo"], np.float32).astype(BF16)
    wg_ = (np.asarray(inputs["w_gate"], np.float32) * nfw[:, :, None]
           ).astype(BF16)
    wu_ = (np.asarray(inputs["w_up"], np.float32) * nfw[:, :, None]
           ).astype(BF16)
    wd_ = np.asarray(inputs["w_down"], np.float32).astype(BF16)
    wout_ = (np.asarray(inputs["w_out"], np.float32)
             * np.asarray(inputs["norm_final_w"], np.float32)[:, None]
             ).astype(BF16)
    rmat_b = np.ascontiguousarray(R.astype(BF16))

    idx_full = np.asarray(inputs["in_idx"]).astype(np.int64)
    tri = (np.arange(128)[:, None] <= np.arange(128)[None, :]).astype(np.float32)
    tri4 = np.tile(tri, (1, 4))
    in_maps = []
    for c in range(C):
        blks = own_blocks(c)
        rs = slice(128 * c, 128 * (c + 1))
        # own-token ids in column order (b, tt): (0,b0),(0,b1),(1,b0),(1,b1)
        ids = np.concatenate([idx_full[b, bl * BS:(bl + 1) * BS]
                              for b in range(B) for bl in blks])
        x0 = np.ascontiguousarray(
            emb[ids].T.reshape(8, 128, OWN).astype(np.float32))
        pos = np.concatenate([np.arange(bl * BS, (bl + 1) * BS) for bl in blks])
        cosT = np.ascontiguousarray(
            np.concatenate([cosT2[:, pos], cosT2[:, pos]], axis=1))
        sinT = np.ascontiguousarray(
            np.concatenate([sinT2[:, pos], sinT2[:, pos]], axis=1))
        mk = np.zeros((16, 128, 512), np.float32)
        for t_, blk in enumerate(blks):
            for jj in range(8):
                j = jj if t_ == 0 else jj + 8
                if j < blk:
                    mk[8 * t_ + jj] = 1.0
                elif j == blk:
                    mk[8 * t_ + jj] = tri4
        qkv = np.ascontiguousarray(np.concatenate(
            [wk_[:, rs, :], wv_[:, rs, :], wq_[:, rs, :]], axis=2))
        wosh = np.ascontiguousarray(wo_[:, rs, :])
        ffn = np.empty((L, 128, 12288), BF16)
        for mb in range(8):
            ffn[:, :, 1024 * mb:1024 * mb + 512] = \
                wg_[:, rs, 512 * mb:512 * (mb + 1)]
            ffn[:, :, 1024 * mb + 512:1024 * (mb + 1)] = \
                wu_[:, rs, 512 * mb:512 * (mb + 1)]
        ffn[:, :, 8192:] = (
            wd_[:, 512 * c:512 * (c + 1), :]
            .reshape(L, 4, 128, D).transpose(0, 2, 1, 3).reshape(L, 128, 4096))
        in_maps.append({
            "x0": x0,
            "cosT": cosT,
            "sinT": sinT,
            "rmat": rmat_b,
            "masks": np.ascontiguousarray(mk.astype(BF16)),
            "qkv_sh": qkv,
            "wo_sh": wosh,
            "ffn_sh": np.ascontiguousarray(ffn),
            "woutc": np.ascontiguousarray(wout_[:, VS * c:VS * (c + 1)]),
        })
    return in_maps


def _assemble(results):
    out = np.empty((B, S, V), np.float32)
    for c in range(C):          # vocab-shard owner
        lg = np.asarray(results[c]["logits"]).astype(np.float32)
        for cp in range(C):     # token owner
            blks = own_blocks(cp)
            for b in range(B):
                for tt in range(2):
                    r0 = cp * 512 + 128 * (2 * b + tt)
                    out[b, blks[tt] * BS:(blks[tt] + 1) * BS,
                        VS * c:VS * (c + 1)] = lg[r0:r0 + 128]
    return out


def run(inputs, trace=False, trace_cores=None):
    nc = _get_nc()
    in_maps = _host_prep(inputs)
    res = run_bass_kernel_spmd(nc, in_maps, list(range(C)), trace=trace,
                               trace_cores=trace_cores)
    return _assemble(res.results), res


def kernel(**inputs):
    out, _ = run(inputs)
    return out


# revision 58
# speedup vs baseline: 1.0139x; 1.0040x over previous
"""Self-contained Trainium2 Bass kernel for the 2-layer Llama3 model.

Sharding: token-sharded compute over 8 cores (core c owns token blocks
{c, 15-c} of each batch, 512 tokens/core), with *minimal host->device
input bytes*: every weight is shipped exactly once, row-sharded by its
contraction dim (1/8 per core), and AllGathered on-device over the
intra-chip links, overlapped with compute. Embedding rows are gathered
on host (2MB/core instead of the 131MB fp32 table). The vocab head is
vocab-sharded: each core keeps its [D, V/8] slice of w_out, final
hidden states are AllGathered (8MB), and each core computes logits for
ALL tokens x its vocab slice, emitted as fp16 (host casts to f32).

Per-core input ~20MB (vs ~257MB replicated), output 32MB fp16.

Device layouts: activations transposed [feature, token] in 128-part
chunks; scores computed transposed [sk, sq]; softmax denominator via a
ones-augmented column on v; RoPE via a +-1 rotation matrix on the PE.
SPMD-uniform program: per-core causality lives in mask *data*.
"""
from contextlib import ExitStack

import numpy as np
import ml_dtypes

import concourse.bass as bass
from concourse.bacc import Bacc
import concourse.mybir as mybir
import concourse.tile as tile
from concourse.bass_utils import run_bass_kernel_spmd

BF16 = ml_dtypes.bfloat16
F32 = mybir.dt.float32
F16 = mybir.dt.float16
BF = mybir.dt.bfloat16

V, D, H, KVH, HD, F, L, B, S = 32000, 1024, 16, 4, 64, 4096, 2, 2, 2048
ROPE_BASE = 500000.0
EPS = 1e-5
SCALE = 1.0 / HD ** 0.5
C = 8            # cores
NB = 16          # 128-token blocks per batch
BS = 128         # block size
OWN = 512        # own tokens per core (2 blocks x 2 batches)
VS = V // C      # vocab shard per core (4000)
NVT = (VS + 511) // 512   # 8 vocab tiles (last is 416 wide)

Exp = mybir.ActivationFunctionType.Exp
Silu = mybir.ActivationFunctionType.Silu
Sqrt = mybir.ActivationFunctionType.Sqrt
Copy = mybir.ActivationFunctionType.Copy
Square = mybir.ActivationFunctionType.Square
MULT = mybir.AluOpType.mult
ADD = mybir.AluOpType.add


def own_blocks(c):
    return [c, NB - 1 - c]


def _colseg(b, t):
    """Own-token column range for (batch b, tile t in {0=low,1=high})."""
    return slice(256 * b + 128 * t, 256 * b + 128 * (t + 1))


# ---------------------------------------------------------------- device ---

def build_nc():
    nc = Bacc()

    x0 = nc.dram_tensor("x0", [8, 128, OWN], BF, kind="ExternalInput")
    cosT = nc.dram_tensor("cosT", [128, OWN], BF, kind="ExternalInput")
    sinT = nc.dram_tensor("sinT", [128, OWN], BF, kind="ExternalInput")
    rmat = nc.dram_tensor("rmat", [128, 128], BF, kind="ExternalInput")
    masks = nc.dram_tensor("masks", [16, 128, 512], BF, kind="ExternalInput")
    # weight shards, row-sharded by contraction dim (1/8 per core):
    # qkv_sh cols: wk 0:256 | wv 256:512 | wq 512:1536
    qkv_sh = nc.dram_tensor("qkv_sh", [L, 128, 1536], BF, kind="ExternalInput")
    wo_sh = nc.dram_tensor("wo_sh", [L, 128, 1024], BF, kind="ExternalInput")
    # ffn_sh cols: 8 groups of [wg 512 | wu 512] (0:8192), then wd
    # as 4 groups of 1024 (F-chunks 4c..4c+3, each [128, D]) at 8192:12288
    ffn_sh = nc.dram_tensor("ffn_sh", [L, 128, 12288], BF, kind="ExternalInput")
    woutc = nc.dram_tensor("woutc", [D, VS], BF, kind="ExternalInput")
    logits = nc.dram_tensor("logits", [C * OWN, VS], F16, kind="ExternalOutput")

    # collective staging (internal) and gathered (Shared) buffers
    qkv_st = nc.dram_tensor("qkv_st", [L, 128, 1536], BF)
    wo_st = nc.dram_tensor("wo_st", [L, 128, 1024], BF)
    ffn_st = nc.dram_tensor("ffn_st", [L, 128, 12288], BF)
    nf_st = nc.dram_tensor("nf_st", [128, 8 * OWN], BF)
    dmy = nc.dram_tensor("dmy", [128, 8], BF)
    dmy_g = nc.dram_tensor("dmyg", [C, 128, 8], BF, addr_space="Shared")
    qkv_g = [nc.dram_tensor(f"qkvg{l}", [C, 128, 1536], BF,
                            addr_space="Shared") for l in range(L)]
    wo_g = [nc.dram_tensor(f"wog{l}", [C, 128, 1024], BF,
                           addr_space="Shared") for l in range(L)]
    ffn_g = [nc.dram_tensor(f"ffng{l}", [C, 128, 12288], BF,
                            addr_space="Shared") for l in range(L)]
    nf_g = nc.dram_tensor("nfg", [C, 128, 8 * OWN], BF, addr_space="Shared")
    kvs = [nc.dram_tensor(f"kvs{l}", [B, 2, 256, 256], BF) for l in range(L)]
    kvr = [nc.dram_tensor(f"kvr{l}", [C, B, 2, 256, 256], BF,
                          addr_space="Shared") for l in range(L)]

    def ag(src_ap, dst_ap):
        nc.gpsimd.collective_compute(
            "AllGather", mybir.AluOpType.bypass,
            replica_groups=[list(range(C))],
            ins=[src_ap], outs=[dst_ap])

    with tile.TileContext(nc) as tc, ExitStack() as st:
        npool = st.enter_context(tc.tile_pool(name="npool", bufs=1))
        sbh = st.enter_context(tc.tile_pool(name="sbh", bufs=2))
        psA = st.enter_context(tc.tile_pool(name="psA", bufs=3, space="PSUM"))
        psB = st.enter_context(tc.tile_pool(name="psB", bufs=2, space="PSUM"))
        psM = st.enter_context(tc.tile_pool(name="psM", bufs=3, space="PSUM"))

        with ExitStack() as body:
            const = body.enter_context(tc.tile_pool(name="const", bufs=1))
            resid = body.enter_context(tc.tile_pool(name="resid", bufs=1))
            qpool = body.enter_context(tc.tile_pool(name="qpool", bufs=1))
            apool = body.enter_context(tc.tile_pool(name="apool", bufs=1))
            hpool = body.enter_context(tc.tile_pool(name="hpool", bufs=1))
            sb = body.enter_context(tc.tile_pool(name="sb", bufs=2))
            wbig = body.enter_context(tc.tile_pool(name="wbig", bufs=2))

            # constants FIRST: memsets live on the gpsimd queue, which the
            # collective waits also occupy — emit before any ag()
            ones_col = const.tile([128, 1], BF, tag="ones_col")
            nc.any.memset(ones_col[:], 1.0)
            ones_row = const.tile([1, 128], BF, tag="ones_row")
            nc.any.memset(ones_row[:], 1.0)
            eps_t = const.tile([1, 1], F32, tag="eps")
            nc.any.memset(eps_t[:], EPS)

            # tiny dummy AllGather to absorb the ~60us collective cold-start
            ag(dmy[:], dmy_g[:])

            # stage layer-0 qkv shard first on the sync ring, then x0
            nc.sync.dma_start(out=qkv_st[0], in_=qkv_sh[0])
            # residual stream xT: 8 chunks [128, OWN] f32, resident
            x = [resid.tile([128, OWN], F32, tag=f"x{k}", name=f"x{k}")
                 for k in range(8)]
            for k in range(8):
                xb = sb.tile([128, OWN], BF, tag="x0b")
                nc.sync.dma_start(out=xb[:], in_=x0[k])
                nc.vector.tensor_copy(out=x[k][:], in_=xb[:])

            t_rmat = const.tile([128, 128], BF, tag="rmat")
            nc.sync.dma_start(out=t_rmat[:], in_=rmat[:])
            t_cos = const.tile([128, OWN], BF, tag="cos")
            nc.sync.dma_start(out=t_cos[:], in_=cosT[:])
            t_sin = const.tile([128, OWN], BF, tag="sin")
            nc.sync.dma_start(out=t_sin[:], in_=sinT[:])

            def ssq_acc(k, ssq):
                """Accumulate sum(x[k]^2) into ssq (psum row)."""
                x2 = sb.tile([128, OWN], BF, tag="x2")
                nc.scalar.activation(out=x2[:], in_=x[k][:], func=Square)
                nc.tensor.matmul(out=ssq[:], lhsT=ones_col[:], rhs=x2[:],
                                 start=(k == 0), stop=(k == 7))

            def rmsnorm(ssq=None):
                """x -> n bf16 chunks (npool tags n0..n7, reused per call)."""
                if ssq is None:
                    ssq = psB.tile([1, OWN], F32, tag="psB")
                    for k in range(8):
                        ssq_acc(k, ssq)
                rms = sbh.tile([1, OWN], F32, tag="rms", bufs=1)
                nc.scalar.activation(out=rms[:], in_=ssq[:], func=Sqrt,
                                     scale=1.0 / D, bias=eps_t[:])
                inv = sbh.tile([1, OWN], F32, tag="inv", bufs=1)
                nc.vector.reciprocal(out=inv[:], in_=rms[:])
                inv_bf = sbh.tile([1, OWN], BF, tag="invbf")
                nc.vector.tensor_copy(out=inv_bf[:], in_=inv[:])
                binv = psB.tile([128, OWN], F32, tag="psB")
                nc.tensor.matmul(out=binv[:], lhsT=ones_row[:], rhs=inv_bf[:],
                                 start=True, stop=True)
                n = [npool.tile([128, OWN], BF, tag=f"n{k}", name=f"n{k}")
                     for k in range(8)]
                for k in range(8):
                    nc.vector.tensor_tensor(out=n[k][:], in0=x[k][:],
                                            in1=binv[:], op=MULT)
                return n

            def rope(pm, y):
                """pm: psum [128, OWN] pre-rope -> bf16 tile y with rope."""
                yr = sb.tile([128, OWN], BF, tag="prerope")
                nc.vector.tensor_copy(out=yr[:], in_=pm[:])
                rot = psA.tile([128, OWN], F32, tag="psA")
                nc.tensor.matmul(out=rot[:], lhsT=t_rmat[:], rhs=yr[:],
                                 start=True, stop=True)
                tmp1 = sb.tile([128, OWN], F32, tag="ropet1", bufs=1)
                nc.vector.tensor_tensor(out=tmp1[:], in0=yr[:], in1=t_cos[:],
                                        op=MULT)
                tmp2 = sb.tile([128, OWN], F32, tag="ropet2", bufs=1)
                nc.vector.tensor_tensor(out=tmp2[:], in0=rot[:], in1=t_sin[:],
                                        op=MULT)
                nc.vector.tensor_tensor(out=y[:], in0=tmp1[:], in1=tmp2[:],
                                        op=ADD)
                return y

            # L0 rmsnorm FIRST so its ACT squares lead the scalar queue,
            # then the first AG, then the remaining staging + mask loads.
            n0 = rmsnorm()
            ag(qkv_st[0], qkv_g[0][:])
            nc.scalar.dma_start(out=wo_st[0], in_=wo_sh[0])
            nc.scalar.dma_start(out=ffn_st[0], in_=ffn_sh[0])
            nc.sync.dma_start(out=qkv_st[1], in_=qkv_sh[1])
            nc.scalar.dma_start(out=wo_st[1], in_=wo_sh[1])
            nc.scalar.dma_start(out=ffn_st[1], in_=ffn_sh[1])
            t_masks = [const.tile([128, 512], BF, tag=f"mask{m}",
                                  name=f"mask{m}") for m in range(16)]
            for m in range(16):
                nc.scalar.dma_start(out=t_masks[m][:], in_=masks[m])

            nssq = [None]
            for l in range(L):
                n = n0 if l == 0 else rmsnorm(ssq=nssq[0])
                nssq[0] = None
                # ---- k/v first so the kv AllGather starts early ----
                wkv = []
                for k in range(8):
                    wt = wbig.tile([128, 512], BF, tag=f"wbig{k}",
                                   name=f"wkv{k}")
                    nc.sync.dma_start(out=wt[:], in_=qkv_g[l][k, :, 0:512])
                    wkv.append(wt)
                kr = [sb.tile([128, OWN], BF, tag=f"kr{mo}", name=f"kr{mo}")
                      for mo in range(2)]
                pk0 = psM.tile([128, OWN], F32, tag="pmm")
                pk1 = psM.tile([128, OWN], F32, tag="pmm")
                for k in range(8):
                    nc.tensor.matmul(out=pk0[:], lhsT=wkv[k][:, 0:128],
                                     rhs=n[k][:], start=(k == 0),
                                     stop=(k == 7))
                    nc.tensor.matmul(out=pk1[:], lhsT=wkv[k][:, 128:256],
                                     rhs=n[k][:], start=(k == 0),
                                     stop=(k == 7))
                rope(pk0, kr[0])
                rope(pk1, kr[1])
                # v natural [own tok, 256]; lhsT = n col-slices
                for tp in range(2):
                    pv0 = psM.tile([128, 256], F32, tag="pmm")
                    pv1 = psM.tile([128, 256], F32, tag="pmm")
                    for k in range(8):
                        nc.tensor.matmul(out=pv0[:],
                                         lhsT=n[k][:, 256 * tp:256 * tp + 128],
                                         rhs=wkv[k][:, 256:512],
                                         start=(k == 0), stop=(k == 7))
                        nc.tensor.matmul(out=pv1[:],
                                         lhsT=n[k][:, 256 * tp + 128:256 * (tp + 1)],
                                         rhs=wkv[k][:, 256:512],
                                         start=(k == 0), stop=(k == 7))
                    for s_, pv in ((0, pv0), (1, pv1)):
                        t_ = 2 * tp + s_
                        vt = sb.tile([128, 256], BF, tag="vnat")
                        nc.vector.tensor_copy(out=vt[:], in_=pv[:])
                        nc.sync.dma_start(
                            out=kvs[l][t_ // 2, 1,
                                       128 * (t_ % 2):128 * (t_ % 2 + 1), :],
                            in_=vt[:])
                for b in range(B):
                    for mo in range(2):
                        nc.sync.dma_start(
                            out=kvs[l][b, 0, 128 * mo:128 * (mo + 1), :],
                            in_=kr[mo][:, 256 * b:256 * (b + 1)])
                ag(kvs[l][:], kvr[l][:])

                # ---- q (overlaps the kv AllGather) ----
                wqt = []
                for k in range(8):
                    wt = wbig.tile([128, 1024], BF, tag=f"wbig{k}",
                                   name=f"wq{k}")
                    nc.sync.dma_start(out=wt[:], in_=qkv_g[l][k, :, 512:1536])
                    wqt.append(wt)
                qr = [qpool.tile([128, OWN], BF, tag=f"qr{mo}", name=f"qr{mo}")
                      for mo in range(8)]
                for mp in range(4):
                    pm0 = psM.tile([128, OWN], F32, tag="pmm")
                    pm1 = psM.tile([128, OWN], F32, tag="pmm")
                    for k in range(8):
                        nc.tensor.matmul(
                            out=pm0[:],
                            lhsT=wqt[k][:, 256 * mp:256 * mp + 128],
                            rhs=n[k][:], start=(k == 0), stop=(k == 7))
                        nc.tensor.matmul(
                            out=pm1[:],
                            lhsT=wqt[k][:, 256 * mp + 128:256 * (mp + 1)],
                            rhs=n[k][:], start=(k == 0), stop=(k == 7))
                    rope(pm0, qr[2 * mp])
                    rope(pm1, qr[2 * mp + 1])

                # queue this layer's wo + ffn AGs behind the kv AG
                ag(wo_st[l], wo_g[l][:])
                ag(ffn_st[l], ffn_g[l][:])
                if l == 0:
                    ag(qkv_st[1], qkv_g[1][:])

                # ---- attention (per batch: assemble k/v, run units) ----
                casm = [apool.tile([128, OWN], BF, tag=f"casm{k}",
                                   name=f"casm{k}") for k in range(8)]
                for b in range(B):
                    kT = [apool.tile([64, S], BF, tag=f"kt{g}", name=f"kt{g}")
                          for g in range(KVH)]
                    for g in range(KVH):
                        src = kvr[l][:, b, 0, 64 * g:64 * (g + 1), :]
                        nc.sync.dma_start(
                            out=kT[g][:, 0:1024].rearrange(
                                "p (r c) -> p r c", r=C),
                            in_=src[:, :, 0:128].transpose([1, 0, 2]))
                        for r in range(C):
                            nc.sync.dma_start(
                                out=kT[g][:, 128 * (NB - 1 - r):128 * (NB - r)],
                                in_=src[r, :, 128:256])
                    v4 = [apool.tile([128, 260], BF, tag=f"v4{j}",
                                     name=f"v4{j}") for j in range(NB)]
                    for j in range(NB):
                        r, i = (j, 0) if j < C else (NB - 1 - j, 1)
                        dst = v4[j][:].rearrange("p (g c) -> p g c", g=4)
                        nc.sync.dma_start(
                            out=dst[:, :, 0:64],
                            in_=kvr[l][r, b, 1, 128 * i:128 * (i + 1), :]
                                .rearrange("p (g c) -> p g c", g=4))
                        nc.any.memset(dst[:, :, 64:65], 1.0)

                    for g in range(KVH):
                        for t in range(2):      # t=0: low block, t=1: high
                            qp = sb.tile([64, 512], BF, tag="qpack")
                            for i in range(4):
                                h = 4 * g + i
                                mo, ro = divmod(h, 2)
                                nc.vector.tensor_copy(
                                    out=qp[:, 128 * i:128 * (i + 1)],
                                    in_=qr[mo][64 * ro:64 * (ro + 1),
                                               _colseg(b, t)])
                            ctx = psB.tile([65, 512], F32, tag="psB")
                            nj = 8 if t == 0 else 16
                            exs = {}

                            def emit_scores(j):
                                sc = psA.tile([128, 512], F32, tag="psA")
                                nc.tensor.matmul(
                                    out=sc[:],
                                    lhsT=kT[g][:, 128 * j:128 * (j + 1)],
                                    rhs=qp[:], start=True, stop=True)
                                ex = sb.tile([128, 512], BF, tag="exp")
                                nc.scalar.activation(out=ex[:], in_=sc[:],
                                                     func=Exp)
                                if t == 0 or j >= 8:
                                    exm = sb.tile([128, 512], BF, tag="expm")
                                    nc.vector.tensor_tensor(
                                        out=exm[:], in0=ex[:],
                                        in1=t_masks[j][:], op=MULT)
                                    ex = exm
                                exs[j] = ex

                            # software-pipeline: scores j+1 issue ahead of
                            # the ctx accumulate of j, so the PE never
                            # waits on the exp/mask round trip
                            emit_scores(0)
                            for j in range(nj):
                                if j + 1 < nj:
                                    emit_scores(j + 1)
                                nc.tensor.matmul(
                                    out=ctx[:],
                                    lhsT=v4[j][:, 65 * g:65 * (g + 1)],
                                    rhs=exs.pop(j)[:], start=(j == 0),
                                    stop=(j == nj - 1))
                            rec = sb.tile([1, 512], F32, tag="rec")
                            nc.vector.reciprocal(out=rec[:], in_=ctx[64:65, :])
                            rec_bf = sb.tile([1, 512], BF, tag="recbf")
                            nc.vector.tensor_copy(out=rec_bf[:], in_=rec[:])
                            brec = psA.tile([64, 512], F32, tag="psA")
                            nc.tensor.matmul(out=brec[:],
                                             lhsT=ones_row[:1, 0:64],
                                             rhs=rec_bf[:], start=True,
                                             stop=True)
                            brec_s = sb.tile([64, 512], BF, tag="brecs")
                            nc.vector.tensor_copy(out=brec_s[:], in_=brec[:])
                            for i in range(4):
                                h = 4 * g + i
                                mo, ro = divmod(h, 2)
                                nc.vector.tensor_tensor(
                                    out=casm[mo][64 * ro:64 * (ro + 1),
                                                 _colseg(b, t)],
                                    in0=ctx[0:64, 128 * i:128 * (i + 1)],
                                    in1=brec_s[:, 128 * i:128 * (i + 1)],
                                    op=MULT)

                # ---- wo + residual ----
                wot = []
                for k in range(8):
                    wt = wbig.tile([128, 1024], BF, tag=f"wbig{k}",
                                   name=f"wo{k}")
                    nc.sync.dma_start(out=wt[:], in_=wo_g[l][k])
                    wot.append(wt)
                ssq2 = psB.tile([1, OWN], F32, tag="psB")
                for mp in range(4):
                    pm0 = psM.tile([128, OWN], F32, tag="pmm")
                    pm1 = psM.tile([128, OWN], F32, tag="pmm")
                    for k in range(8):
                        nc.tensor.matmul(
                            out=pm0[:],
                            lhsT=wot[k][:, 256 * mp:256 * mp + 128],
                            rhs=casm[k][:], start=(k == 0), stop=(k == 7))
                        nc.tensor.matmul(
                            out=pm1[:],
                            lhsT=wot[k][:, 256 * mp + 128:256 * (mp + 1)],
                            rhs=casm[k][:], start=(k == 0), stop=(k == 7))
                    nc.vector.tensor_tensor(out=x[2 * mp][:], in0=x[2 * mp][:],
                                            in1=pm0[:], op=ADD)
                    ssq_acc(2 * mp, ssq2)
                    nc.vector.tensor_tensor(out=x[2 * mp + 1][:],
                                            in0=x[2 * mp + 1][:],
                                            in1=pm1[:], op=ADD)
                    ssq_acc(2 * mp + 1, ssq2)

                # ---- FFN ----
                n2 = rmsnorm(ssq=ssq2)
                ht = [hpool.tile([128, OWN], BF, tag=f"h{mo}", name=f"h{mo}")
                      for mo in range(32)]
                for mb in range(8):
                    wgu = []
                    for k in range(8):
                        a = wbig.tile([128, 1024], BF, tag=f"wbig{k}",
                                      name=f"wgu{k}")
                        nc.sync.dma_start(
                            out=a[:],
                            in_=ffn_g[l][k, :, 1024 * mb:1024 * (mb + 1)])
                        wgu.append(a)
                    for ms in range(4):
                        mo = 4 * mb + ms
                        pg = psM.tile([128, OWN], F32, tag="pmm")
                        pu = psM.tile([128, OWN], F32, tag="pmm")
                        for k in range(8):
                            nc.tensor.matmul(
                                out=pg[:],
                                lhsT=wgu[k][:, 128 * ms:128 * (ms + 1)],
                                rhs=n2[k][:], start=(k == 0), stop=(k == 7))
                            nc.tensor.matmul(
                                out=pu[:],
                                lhsT=wgu[k][:, 512 + 128 * ms:512 + 128 * (ms + 1)],
                                rhs=n2[k][:], start=(k == 0), stop=(k == 7))
                        gs = sb.tile([128, OWN], BF, tag="gsilu")
                        nc.scalar.activation(out=gs[:], in_=pg[:], func=Silu)
                        nc.vector.tensor_tensor(out=ht[mo][:], in0=pu[:],
                                                in1=gs[:], op=MULT)
                # down-proj: two output chunks per pass, stream wd tiles
                ssq3 = psB.tile([1, OWN], F32, tag="psB")
                for mp in range(4):
                    pd0 = psM.tile([128, OWN], F32, tag="pmm")
                    pd1 = psM.tile([128, OWN], F32, tag="pmm")
                    for kk in range(32):
                        c_, j = divmod(kk, 4)
                        wt = wbig.tile([128, 256], BF, tag="wsm", bufs=4,
                                       name="wdt")
                        eng = nc.sync if kk % 2 == 0 else nc.scalar
                        eng.dma_start(
                            out=wt[:],
                            in_=ffn_g[l][c_, :, 8192 + 1024 * j + 256 * mp:
                                         8192 + 1024 * j + 256 * (mp + 1)])
                        nc.tensor.matmul(out=pd0[:], lhsT=wt[:, 0:128],
                                         rhs=ht[kk][:], start=(kk == 0),
                                         stop=(kk == 31))
                        nc.tensor.matmul(out=pd1[:], lhsT=wt[:, 128:256],
                                         rhs=ht[kk][:], start=(kk == 0),
                                         stop=(kk == 31))
                    nc.vector.tensor_tensor(out=x[2 * mp][:], in0=x[2 * mp][:],
                                            in1=pd0[:], op=ADD)
                    nc.vector.tensor_tensor(out=x[2 * mp + 1][:],
                                            in0=x[2 * mp + 1][:],
                                            in1=pd1[:], op=ADD)
                    ssq_acc(2 * mp, ssq3)
                    ssq_acc(2 * mp + 1, ssq3)
                nssq[0] = ssq3

            # ---- final norm -> nf (npool, survives body pools) ----
            nf = rmsnorm(ssq=nssq[0])
            for k in range(8):
                nc.sync.dma_start(out=nf_st[:, 512 * k:512 * (k + 1)],
                                  in_=nf[k][:])
            ag(nf_st[:], nf_g[:])

        # ---- vocab-sharded head: all tokens x our V/8 slice ----
        with ExitStack() as hd:
            hp = hd.enter_context(tc.tile_pool(name="hp", bufs=1))
            hw = hd.enter_context(tc.tile_pool(name="hw", bufs=2))
            whead = []
            for k in range(8):
                wt = hp.tile([128, VS], BF, tag=f"wh{k}", name=f"wh{k}")
                nc.sync.dma_start(out=wt[:], in_=woutc[128 * k:128 * (k + 1), :])
                whead.append(wt)
            for cp in range(C):
                nfo = hw.tile([128, 8 * OWN], BF, tag="nfo")
                nc.sync.dma_start(out=nfo[:], in_=nf_g[cp])
                for tb in range(4):
                    for vp in range(NVT // 2):
                        vt0, vt1 = 2 * vp, 2 * vp + 1
                        vw1 = min(512, VS - 512 * vt1)
                        ph0 = psM.tile([128, 512], F32, tag="pmm")
                        ph1 = psM.tile([128, 512], F32, tag="pmm")
                        for k in range(8):
                            lhs = nfo[:, 512 * k + 128 * tb:
                                      512 * k + 128 * (tb + 1)]
                            nc.tensor.matmul(
                                out=ph0[:], lhsT=lhs,
                                rhs=whead[k][:, 512 * vt0:512 * (vt0 + 1)],
                                start=(k == 0), stop=(k == 7))
                            nc.tensor.matmul(
                                out=ph1[:, :vw1], lhsT=lhs,
                                rhs=whead[k][:, 512 * vt1:512 * vt1 + vw1],
                                start=(k == 0), stop=(k == 7))
                        for vt, vw, ph, eng in ((vt0, 512, ph0, 0),
                                                (vt1, vw1, ph1, 1)):
                            ot = hw.tile([128, 512], F16, tag="hout")
                            if eng == 0:
                                nc.vector.tensor_copy(out=ot[:, :vw],
                                                      in_=ph[:, :vw])
                            else:
                                nc.scalar.activation(out=ot[:, :vw],
                                                     in_=ph[:, :vw],
                                                     func=Copy)
                            nc.sync.dma_start(
                                out=logits[512 * cp + 128 * tb:
                                           512 * cp + 128 * (tb + 1),
                                           512 * vt:512 * vt + vw],
                                in_=ot[:, :vw])

    return nc


# ------------------------------------------------------------------ host ---

_NC_CACHE = {}


def _get_nc():
    if "nc" not in _NC_CACHE:
        nc = build_nc()
        nc.finalize()
        _NC_CACHE["nc"] = nc
    return _NC_CACHE["nc"]


def _host_prep(inputs):
    inv_freq = 1.0 / ROPE_BASE ** (np.arange(0, HD, 2, dtype=np.float32) / HD)
    t = np.arange(S, dtype=np.float32)
    freqs = t[:, None] * inv_freq[None, :]
    ang = np.concatenate([freqs, freqs], axis=-1)       # [S, 64]
    cos_full, sin_full = np.cos(ang), np.sin(ang)
    cosT2 = np.empty((128, S), np.float32)
    sinT2 = np.empty((128, S), np.float32)
    for p in range(128):
        d = p % 64
        cosT2[p] = cos_full[:, d]
        sinT2[p] = sin_full[:, d] * (-1.0 if d < 32 else 1.0)

    R = np.zeros((128, 128), np.float32)
    for blk in range(2):
        o = blk * 64
        for j in range(32):
            R[o + 32 + j, o + j] = 1.0
            R[o + j, o + 32 + j] = 1.0

    naw = np.asarray(inputs["norm_attn_w"], np.float32)
    nfw = np.asarray(inputs["norm_ff_w"], np.float32)
    emb = np.asarray(inputs["token_emb"], np.float32)
    wq_ = (np.asarray(inputs["wq"], np.float32) * naw[:, :, None] * SCALE
           ).astype(BF16)
    wk_ = (np.asarray(inputs["wk"], np.float32) * naw[:, :, None]).astype(BF16)
    wv_ = (np.asarray(inputs["wv"], np.float32) * naw[:, :, None]).astype(BF16)
    wo_ = np.asarray(inputs["wo"], np.float32).astype(BF16)
    wg_ = (np.asarray(inputs["w_gate"], np.float32) * nfw[:, :, None]
           ).astype(BF16)
    wu_ = (np.asarray(inputs["w_up"], np.float32) * nfw[:, :, None]
           ).astype(BF16)
    wd_ = np.asarray(inputs["w_down"], np.float32).astype(BF16)
    wout_ = (np.asarray(inputs["w_out"], np.float32)
             * np.asarray(inputs["norm_final_w"], np.float32)[:, None]
             ).astype(BF16)
    rmat_b = np.ascontiguousarray(R.astype(BF16))

    idx_full = np.asarray(inputs["in_idx"]).astype(np.int64)
    tri = (np.arange(128)[:, None] <= np.arange(128)[None, :]).astype(np.float32)
    tri4 = np.tile(tri, (1, 4))
    in_maps = []
    for c in range(C):
        blks = own_blocks(c)
        rs = slice(128 * c, 128 * (c + 1))
        # own-token ids in column order (b, tt): (0,b0),(0,b1),(1,b0),(1,b1)
        ids = np.concatenate([idx_full[b, bl * BS:(bl + 1) * BS]
                              for b in range(B) for bl in blks])
        x0 = np.ascontiguousarray(
            emb[ids].T.reshape(8, 128, OWN).astype(BF16))
        pos = np.concatenate([np.arange(bl * BS, (bl + 1) * BS) for bl in blks])
        cosT = np.ascontiguousarray(
            np.concatenate([cosT2[:, pos], cosT2[:, pos]], axis=1).astype(BF16))
        sinT = np.ascontiguousarray(
            np.concatenate([sinT2[:, pos], sinT2[:, pos]], axis=1).astype(BF16))
        mk = np.zeros((16, 128, 512), np.float32)
        for t_, blk in enumerate(blks):
            for jj in range(8):
                j = jj if t_ == 0 else jj + 8
                if j < blk:
                    mk[8 * t_ + jj] = 1.0
                elif j == blk:
                    mk[8 * t_ + jj] = tri4
        qkv = np.ascontiguousarray(np.concatenate(
            [wk_[:, rs, :], wv_[:, rs, :], wq_[:, rs, :]], axis=2))
        wosh = np.ascontiguousarray(wo_[:, rs, :])
        ffn = np.empty((L, 128, 12288), BF16)
        for mb in range(8):
            ffn[:, :, 1024 * mb:1024 * mb + 512] = \
                wg_[:, rs, 512 * mb:512 * (mb + 1)]
            ffn[:, :, 1024 * mb + 512:1024 * (mb + 1)] = \
                wu_[:, rs, 512 * mb:512 * (mb + 1)]
        ffn[:, :, 8192:] = (
            wd_[:, 512 * c:512 * (c + 1), :]
            .reshape(L, 4, 128, D).transpose(0, 2, 1, 3).reshape(L, 128, 4096))
        in_maps.append({
            "x0": x0,
            "cosT": cosT,
            "sinT": sinT,
            "rmat": rmat_b,
            "masks": np.ascontiguousarray(mk.astype(BF16)),
            "qkv_sh": qkv,
            "wo_sh": wosh,
            "ffn_sh": np.ascontiguousarray(ffn),
            "woutc": np.ascontiguousarray(wout_[:, VS * c:VS * (c + 1)]),
        })
    return in_maps


def _assemble(results):
    out = np.empty((B, S, V), np.float32)
    for c in range(C):          # vocab-shard owner
        lg = np.asarray(results[c]["logits"]).astype(np.float32)
        for cp in range(C):     # token owner
            blks = own_blocks(cp)
            for b in range(B):
                for tt in range(2):
                    r0 = cp * 512 + 128 * (2 * b + tt)
                    out[b, blks[tt] * BS:(blks[tt] + 1) * BS,
                        VS * c:VS * (c + 1)] = lg[r0:r0 + 128]
    return out


def run(inputs, trace=False, trace_cores=None):
    nc = _get_nc()
    in_maps = _host_prep(inputs)
    res = run_bass_kernel_spmd(nc, in_maps, list(range(C)), trace=trace,
                               trace_cores=trace_cores)
    return _assemble(res.results), res


def kernel(**inputs):
    out, _ = run(inputs)
    return out


# revision 64
# speedup vs baseline: 1.0353x; 1.0211x over previous
"""Self-contained Trainium2 Bass kernel for the 2-layer Llama3 model.

Sharding: token-sharded compute over 8 cores (core c owns token blocks
{c, 15-c} of each batch, 512 tokens/core), with *minimal host->device
input bytes*: every weight is shipped exactly once, row-sharded by its
contraction dim (1/8 per core), and AllGathered on-device over the
intra-chip links, overlapped with compute. Embedding rows are gathered
on host (2MB/core instead of the 131MB fp32 table). The vocab head is
vocab-sharded: each core keeps its [D, V/8] slice of w_out, final
hidden states are AllGathered (8MB), and each core computes logits for
ALL tokens x its vocab slice, emitted as fp16 (host casts to f32).

Per-core input ~20MB (vs ~257MB replicated), output 32MB fp16.

Device layouts: activations transposed [feature, token] in 128-part
chunks; scores computed transposed [sk, sq]; softmax denominator via a
ones-augmented column on v; RoPE via a +-1 rotation matrix on the PE.
SPMD-uniform program: per-core causality lives in mask *data*.
"""
from contextlib import ExitStack

import numpy as np
import ml_dtypes

import concourse.bass as bass
from concourse.bacc import Bacc
import concourse.mybir as mybir
import concourse.tile as tile
from concourse.bass_utils import run_bass_kernel_spmd

BF16 = ml_dtypes.bfloat16
F32 = mybir.dt.float32
F16 = mybir.dt.float16
BF = mybir.dt.bfloat16

V, D, H, KVH, HD, F, L, B, S = 32000, 1024, 16, 4, 64, 4096, 2, 2, 2048
ROPE_BASE = 500000.0
EPS = 1e-5
SCALE = 1.0 / HD ** 0.5
C = 8            # cores
NB = 16          # 128-token blocks per batch
BS = 128         # block size
OWN = 512        # own tokens per core (2 blocks x 2 batches)
VS = V // C      # vocab shard per core (4000)
NVT = (VS + 511) // 512   # 8 vocab tiles (last is 416 wide)

Exp = mybir.ActivationFunctionType.Exp
Silu = mybir.ActivationFunctionType.Silu
Sqrt = mybir.ActivationFunctionType.Sqrt
Copy = mybir.ActivationFunctionType.Copy
Square = mybir.ActivationFunctionType.Square
MULT = mybir.AluOpType.mult
ADD = mybir.AluOpType.add


def own_blocks(c):
    return [c, NB - 1 - c]


def _colseg(b, t):
    """Own-token column range for (batch b, tile t in {0=low,1=high})."""
    return slice(256 * b + 128 * t, 256 * b + 128 * (t + 1))


# ---------------------------------------------------------------- device ---

def build_nc():
    nc = Bacc()

    x0 = nc.dram_tensor("x0", [8, 128, OWN], BF, kind="ExternalInput")
    cosT = nc.dram_tensor("cosT", [128, OWN], BF, kind="ExternalInput")
    sinT = nc.dram_tensor("sinT", [128, OWN], BF, kind="ExternalInput")
    rmat = nc.dram_tensor("rmat", [128, 128], BF, kind="ExternalInput")
    masks = nc.dram_tensor("masks", [16, 128, 512], BF, kind="ExternalInput")
    # weight shards, row-sharded by contraction dim (1/8 per core):
    # qkv_sh cols: wk 0:256 | wv 256:512 | wq 512:1536
    qkv_sh = nc.dram_tensor("qkv_sh", [L, 128, 1536], BF, kind="ExternalInput")
    wo_sh = nc.dram_tensor("wo_sh", [L, 128, 1024], BF, kind="ExternalInput")
    # ffn_sh cols: 8 groups of [wg 512 | wu 512] (0:8192), then wd
    # as 4 groups of 1024 (F-chunks 4c..4c+3, each [128, D]) at 8192:12288
    ffn_sh = nc.dram_tensor("ffn_sh", [L, 128, 12288], BF, kind="ExternalInput")
    woutc = nc.dram_tensor("woutc", [D, VS], BF, kind="ExternalInput")
    logits = nc.dram_tensor("logits", [C * OWN, VS], F16, kind="ExternalOutput")

    # collective staging (internal) and gathered (Shared) buffers
    qkv_st = nc.dram_tensor("qkv_st", [L, 128, 1536], BF)
    wo_st = nc.dram_tensor("wo_st", [L, 128, 1024], BF)
    ffn_st = nc.dram_tensor("ffn_st", [L, 128, 12288], BF)
    nf_st = nc.dram_tensor("nf_st", [128, 8 * OWN], BF)
    dmy = nc.dram_tensor("dmy", [128, 8], BF)
    dmy_g = nc.dram_tensor("dmyg", [C, 128, 8], BF, addr_space="Shared")
    qkv_g = [nc.dram_tensor(f"qkvg{l}", [C, 128, 1536], BF,
                            addr_space="Shared") for l in range(L)]
    wo_g = [nc.dram_tensor(f"wog{l}", [C, 128, 1024], BF,
                           addr_space="Shared") for l in range(L)]
    ffn_g = [nc.dram_tensor(f"ffng{l}", [C, 128, 12288], BF,
                            addr_space="Shared") for l in range(L)]
    nf_g = nc.dram_tensor("nfg", [C, 128, 8 * OWN], BF, addr_space="Shared")
    kvs = [nc.dram_tensor(f"kvs{l}", [B, 2, 256, 256], BF) for l in range(L)]
    kvr = [nc.dram_tensor(f"kvr{l}", [C, B, 2, 256, 256], BF,
                          addr_space="Shared") for l in range(L)]

    def ag(src_ap, dst_ap):
        nc.gpsimd.collective_compute(
            "AllGather", mybir.AluOpType.bypass,
            replica_groups=[list(range(C))],
            ins=[src_ap], outs=[dst_ap])

    with tile.TileContext(nc) as tc, ExitStack() as st:
        npool = st.enter_context(tc.tile_pool(name="npool", bufs=1))
        sbh = st.enter_context(tc.tile_pool(name="sbh", bufs=2))
        # psM slots are [128,1024] f32 (2 banks); 3 bufs + psB 2 = 8 banks
        psB = st.enter_context(tc.tile_pool(name="psB", bufs=2, space="PSUM"))
        psM = st.enter_context(tc.tile_pool(name="psM", bufs=3, space="PSUM"))

        with ExitStack() as body:
            const = body.enter_context(tc.tile_pool(name="const", bufs=1))
            resid = body.enter_context(tc.tile_pool(name="resid", bufs=1))
            qpool = body.enter_context(tc.tile_pool(name="qpool", bufs=1))
            apool = body.enter_context(tc.tile_pool(name="apool", bufs=1))
            hpool = body.enter_context(tc.tile_pool(name="hpool", bufs=1))
            sb = body.enter_context(tc.tile_pool(name="sb", bufs=2))
            wbig = body.enter_context(tc.tile_pool(name="wbig", bufs=2))

            # constants FIRST: memsets live on the gpsimd queue, which the
            # collective waits also occupy — emit before any ag()
            ones_col = const.tile([128, 1], BF, tag="ones_col")
            nc.any.memset(ones_col[:], 1.0)
            ones_row = const.tile([1, 128], BF, tag="ones_row")
            nc.any.memset(ones_row[:], 1.0)
            eps_t = const.tile([1, 1], F32, tag="eps")
            nc.any.memset(eps_t[:], EPS)

            # tiny dummy AllGather to absorb the ~60us collective cold-start
            ag(dmy[:], dmy_g[:])

            # stage layer-0 qkv shard first on the sync ring, then x0
            nc.sync.dma_start(out=qkv_st[0], in_=qkv_sh[0])
            # residual stream xT: 8 chunks [128, OWN] f32, resident
            x = [resid.tile([128, OWN], F32, tag=f"x{k}", name=f"x{k}")
                 for k in range(8)]
            x0b = sb.tile([128, 8 * OWN], BF, tag="x0b", bufs=1)
            nc.sync.dma_start(out=x0b[:].rearrange("p (k c) -> p k c", k=8),
                              in_=x0[:].transpose([1, 0, 2]))
            for k in range(8):
                nc.vector.tensor_copy(out=x[k][:],
                                      in_=x0b[:, 512 * k:512 * (k + 1)])

            t_rmat = const.tile([128, 128], BF, tag="rmat")
            nc.sync.dma_start(out=t_rmat[:], in_=rmat[:])
            t_cos = const.tile([128, OWN], BF, tag="cos")
            nc.sync.dma_start(out=t_cos[:], in_=cosT[:])
            t_sin = const.tile([128, OWN], BF, tag="sin")
            nc.sync.dma_start(out=t_sin[:], in_=sinT[:])

            def ssq_acc(k, ssq):
                """Accumulate sum(x[k]^2) into ssq (psum row)."""
                x2 = sb.tile([128, OWN], BF, tag="x2")
                nc.scalar.activation(out=x2[:], in_=x[k][:], func=Square)
                nc.tensor.matmul(out=ssq[:], lhsT=ones_col[:], rhs=x2[:],
                                 start=(k == 0), stop=(k == 7))

            def rmsnorm(ssq=None):
                """x -> n bf16 chunks (npool tags n0..n7, reused per call)."""
                if ssq is None:
                    ssq = psB.tile([1, OWN], F32, tag="psB")
                    for k in range(8):
                        ssq_acc(k, ssq)
                rms = sbh.tile([1, OWN], F32, tag="rms", bufs=1)
                nc.scalar.activation(out=rms[:], in_=ssq[:], func=Sqrt,
                                     scale=1.0 / D, bias=eps_t[:])
                inv = sbh.tile([1, OWN], F32, tag="inv", bufs=1)
                nc.vector.reciprocal(out=inv[:], in_=rms[:])
                inv_bf = sbh.tile([1, OWN], BF, tag="invbf")
                nc.vector.tensor_copy(out=inv_bf[:], in_=inv[:])
                binv = psB.tile([128, OWN], F32, tag="psB")
                nc.tensor.matmul(out=binv[:], lhsT=ones_row[:], rhs=inv_bf[:],
                                 start=True, stop=True)
                n = [npool.tile([128, OWN], BF, tag=f"n{k}", name=f"n{k}")
                     for k in range(8)]
                for k in range(8):
                    nc.vector.tensor_tensor(out=n[k][:], in0=x[k][:],
                                            in1=binv[:], op=MULT)
                return n

            def rope(pm, y):
                """pm: psum [128, OWN] pre-rope -> bf16 tile y with rope."""
                yr = sb.tile([128, OWN], BF, tag="prerope")
                nc.vector.tensor_copy(out=yr[:], in_=pm[:])
                rot = psM.tile([128, OWN], F32, tag="pmm")
                nc.tensor.matmul(out=rot[:], lhsT=t_rmat[:], rhs=yr[:],
                                 start=True, stop=True)
                tmp1 = sb.tile([128, OWN], F32, tag="ropet1", bufs=1)
                nc.vector.tensor_tensor(out=tmp1[:], in0=yr[:], in1=t_cos[:],
                                        op=MULT)
                tmp2 = sb.tile([128, OWN], F32, tag="ropet2", bufs=1)
                nc.vector.tensor_tensor(out=tmp2[:], in0=rot[:], in1=t_sin[:],
                                        op=MULT)
                nc.vector.tensor_tensor(out=y[:], in0=tmp1[:], in1=tmp2[:],
                                        op=ADD)
                return y

            # L0 rmsnorm FIRST so its ACT squares lead the scalar queue,
            # then the first AG, then the remaining staging + mask loads.
            n0 = rmsnorm()
            ag(qkv_st[0], qkv_g[0][:])
            nc.scalar.dma_start(out=wo_st[0], in_=wo_sh[0])
            nc.scalar.dma_start(out=ffn_st[0], in_=ffn_sh[0])
            nc.sync.dma_start(out=qkv_st[1], in_=qkv_sh[1])
            nc.scalar.dma_start(out=wo_st[1], in_=wo_sh[1])
            nc.scalar.dma_start(out=ffn_st[1], in_=ffn_sh[1])
            t_masks = const.tile([128, 16 * 512], BF, tag="masks")
            nc.scalar.dma_start(
                out=t_masks[:].rearrange("p (m c) -> p m c", m=16),
                in_=masks[:].transpose([1, 0, 2]))

            nssq = [None]
            for l in range(L):
                n = n0 if l == 0 else rmsnorm(ssq=nssq[0])
                nssq[0] = None
                # ---- k/v first so the kv AllGather starts early ----
                wkv = []
                for k in range(8):
                    wt = wbig.tile([128, 512], BF, tag=f"wbig{k}",
                                   name=f"wkv{k}")
                    nc.sync.dma_start(out=wt[:], in_=qkv_g[l][k, :, 0:512])
                    wkv.append(wt)
                kr = [sb.tile([128, OWN], BF, tag=f"kr{mo}", name=f"kr{mo}")
                      for mo in range(2)]
                pk0 = psM.tile([128, OWN], F32, tag="pmm")
                pk1 = psM.tile([128, OWN], F32, tag="pmm")
                for k in range(8):
                    nc.tensor.matmul(out=pk0[:], lhsT=wkv[k][:, 0:128],
                                     rhs=n[k][:], start=(k == 0),
                                     stop=(k == 7))
                    nc.tensor.matmul(out=pk1[:], lhsT=wkv[k][:, 128:256],
                                     rhs=n[k][:], start=(k == 0),
                                     stop=(k == 7))
                rope(pk0, kr[0])
                rope(pk1, kr[1])
                # v natural [own tok, 256]; lhsT = n col-slices
                for tp in range(2):
                    pv0 = psM.tile([128, 256], F32, tag="pmm")
                    pv1 = psM.tile([128, 256], F32, tag="pmm")
                    for k in range(8):
                        nc.tensor.matmul(out=pv0[:],
                                         lhsT=n[k][:, 256 * tp:256 * tp + 128],
                                         rhs=wkv[k][:, 256:512],
                                         start=(k == 0), stop=(k == 7))
                        nc.tensor.matmul(out=pv1[:],
                                         lhsT=n[k][:, 256 * tp + 128:256 * (tp + 1)],
                                         rhs=wkv[k][:, 256:512],
                                         start=(k == 0), stop=(k == 7))
                    for s_, pv in ((0, pv0), (1, pv1)):
                        t_ = 2 * tp + s_
                        vt = sb.tile([128, 256], BF, tag="vnat")
                        nc.vector.tensor_copy(out=vt[:], in_=pv[:])
                        nc.sync.dma_start(
                            out=kvs[l][t_ // 2, 1,
                                       128 * (t_ % 2):128 * (t_ % 2 + 1), :],
                            in_=vt[:])
                for b in range(B):
                    for mo in range(2):
                        nc.sync.dma_start(
                            out=kvs[l][b, 0, 128 * mo:128 * (mo + 1), :],
                            in_=kr[mo][:, 256 * b:256 * (b + 1)])
                ag(kvs[l][:], kvr[l][:])

                # ---- q (overlaps the kv AllGather) ----
                wqt = []
                for k in range(8):
                    wt = wbig.tile([128, 1024], BF, tag=f"wbig{k}",
                                   name=f"wq{k}")
                    nc.sync.dma_start(out=wt[:], in_=qkv_g[l][k, :, 512:1536])
                    wqt.append(wt)
                qr = [qpool.tile([128, OWN], BF, tag=f"qr{mo}", name=f"qr{mo}")
                      for mo in range(8)]
                for mp in range(4):
                    pm0 = psM.tile([128, OWN], F32, tag="pmm")
                    pm1 = psM.tile([128, OWN], F32, tag="pmm")
                    for k in range(8):
                        nc.tensor.matmul(
                            out=pm0[:],
                            lhsT=wqt[k][:, 256 * mp:256 * mp + 128],
                            rhs=n[k][:], start=(k == 0), stop=(k == 7))
                        nc.tensor.matmul(
                            out=pm1[:],
                            lhsT=wqt[k][:, 256 * mp + 128:256 * (mp + 1)],
                            rhs=n[k][:], start=(k == 0), stop=(k == 7))
                    rope(pm0, qr[2 * mp])
                    rope(pm1, qr[2 * mp + 1])

                # queue this layer's wo + ffn AGs behind the kv AG
                ag(wo_st[l], wo_g[l][:])
                ag(ffn_st[l], ffn_g[l][:])
                if l == 0:
                    ag(qkv_st[1], qkv_g[1][:])

                # ---- attention (per batch: assemble k/v, run units) ----
                casm = [apool.tile([128, OWN], BF, tag=f"casm{k}",
                                   name=f"casm{k}") for k in range(8)]
                for b in range(B):
                    kT = [apool.tile([64, S], BF, tag=f"kt{g}", name=f"kt{g}")
                          for g in range(KVH)]
                    for g in range(KVH):
                        src = kvr[l][:, b, 0, 64 * g:64 * (g + 1), :]
                        nc.sync.dma_start(
                            out=kT[g][:, 0:1024].rearrange(
                                "p (r c) -> p r c", r=C),
                            in_=src[:, :, 0:128].transpose([1, 0, 2]))
                        for r in range(C):
                            nc.sync.dma_start(
                                out=kT[g][:, 128 * (NB - 1 - r):128 * (NB - r)],
                                in_=src[r, :, 128:256])
                    v4 = [apool.tile([128, 260], BF, tag=f"v4{j}",
                                     name=f"v4{j}") for j in range(NB)]
                    for j in range(NB):
                        r, i = (j, 0) if j < C else (NB - 1 - j, 1)
                        dst = v4[j][:].rearrange("p (g c) -> p g c", g=4)
                        nc.sync.dma_start(
                            out=dst[:, :, 0:64],
                            in_=kvr[l][r, b, 1, 128 * i:128 * (i + 1), :]
                                .rearrange("p (g c) -> p g c", g=4))
                        nc.any.memset(dst[:, :, 64:65], 1.0)

                    for g in range(KVH):
                        for t in range(2):      # t=0: low block, t=1: high
                            qp = sb.tile([64, 512], BF, tag="qpack")
                            for i in range(4):
                                h = 4 * g + i
                                mo, ro = divmod(h, 2)
                                nc.vector.tensor_copy(
                                    out=qp[:, 128 * i:128 * (i + 1)],
                                    in_=qr[mo][64 * ro:64 * (ro + 1),
                                               _colseg(b, t)])
                            ctx = psB.tile([65, 512], F32, tag="psB")
                            nj = 8 if t == 0 else 16
                            npair = nj // 2
                            exs = {}

                            def emit_pair(p):
                                # two key-chunks share a 2-bank psum tile ->
                                # one exp + one mask mult per 1024 cols
                                scp = psM.tile([128, 1024], F32, tag="pmm")
                                for s_ in range(2):
                                    j = 2 * p + s_
                                    nc.tensor.matmul(
                                        out=scp[:, 512 * s_:512 * (s_ + 1)],
                                        lhsT=kT[g][:, 128 * j:128 * (j + 1)],
                                        rhs=qp[:], start=True, stop=True)
                                ex = sb.tile([128, 1024], BF, tag="exp")
                                nc.scalar.activation(out=ex[:], in_=scp[:],
                                                     func=Exp)
                                if t == 0 or p >= 4:
                                    exm = sb.tile([128, 1024], BF, tag="expm")
                                    nc.vector.tensor_tensor(
                                        out=exm[:], in0=ex[:],
                                        in1=t_masks[:, 512 * 2 * p:
                                                    512 * 2 * (p + 1)],
                                        op=MULT)
                                    ex = exm
                                exs[p] = ex

                            # software-pipeline: pair p+1's scores issue
                            # ahead of pair p's ctx accumulates
                            emit_pair(0)
                            for p in range(npair):
                                if p + 1 < npair:
                                    emit_pair(p + 1)
                                ex = exs.pop(p)
                                for s_ in range(2):
                                    j = 2 * p + s_
                                    nc.tensor.matmul(
                                        out=ctx[:],
                                        lhsT=v4[j][:, 65 * g:65 * (g + 1)],
                                        rhs=ex[:, 512 * s_:512 * (s_ + 1)],
                                        start=(j == 0), stop=(j == nj - 1))
                            rec = sb.tile([1, 512], F32, tag="rec")
                            nc.vector.reciprocal(out=rec[:], in_=ctx[64:65, :])
                            rec_bf = sb.tile([1, 512], BF, tag="recbf")
                            nc.vector.tensor_copy(out=rec_bf[:], in_=rec[:])
                            brec = psM.tile([64, 512], F32, tag="pmm")
                            nc.tensor.matmul(out=brec[:],
                                             lhsT=ones_row[:1, 0:64],
                                             rhs=rec_bf[:], start=True,
                                             stop=True)
                            brec_s = sb.tile([64, 512], BF, tag="brecs")
                            nc.vector.tensor_copy(out=brec_s[:], in_=brec[:])
                            for i in range(4):
                                h = 4 * g + i
                                mo, ro = divmod(h, 2)
                                nc.vector.tensor_tensor(
                                    out=casm[mo][64 * ro:64 * (ro + 1),
                                                 _colseg(b, t)],
                                    in0=ctx[0:64, 128 * i:128 * (i + 1)],
                                    in1=brec_s[:, 128 * i:128 * (i + 1)],
                                    op=MULT)

                # ---- wo + residual ----
                wot = []
                for k in range(8):
                    wt = wbig.tile([128, 1024], BF, tag=f"wbig{k}",
                                   name=f"wo{k}")
                    nc.sync.dma_start(out=wt[:], in_=wo_g[l][k])
                    wot.append(wt)
                ssq2 = psB.tile([1, OWN], F32, tag="psB")
                for mp in range(4):
                    pm0 = psM.tile([128, OWN], F32, tag="pmm")
                    pm1 = psM.tile([128, OWN], F32, tag="pmm")
                    for k in range(8):
                        nc.tensor.matmul(
                            out=pm0[:],
                            lhsT=wot[k][:, 256 * mp:256 * mp + 128],
                            rhs=casm[k][:], start=(k == 0), stop=(k == 7))
                        nc.tensor.matmul(
                            out=pm1[:],
                            lhsT=wot[k][:, 256 * mp + 128:256 * (mp + 1)],
                            rhs=casm[k][:], start=(k == 0), stop=(k == 7))
                    nc.vector.tensor_tensor(out=x[2 * mp][:], in0=x[2 * mp][:],
                                            in1=pm0[:], op=ADD)
                    ssq_acc(2 * mp, ssq2)
                    nc.vector.tensor_tensor(out=x[2 * mp + 1][:],
                                            in0=x[2 * mp + 1][:],
                                            in1=pm1[:], op=ADD)
                    ssq_acc(2 * mp + 1, ssq2)

                # ---- FFN ----
                n2 = rmsnorm(ssq=ssq2)
                ht = [hpool.tile([128, OWN], BF, tag=f"h{mo}", name=f"h{mo}")
                      for mo in range(32)]
                for mb in range(8):
                    wgu = []
                    for k in range(8):
                        a = wbig.tile([128, 1024], BF, tag=f"wbig{k}",
                                      name=f"wgu{k}")
                        nc.sync.dma_start(
                            out=a[:],
                            in_=ffn_g[l][k, :, 1024 * mb:1024 * (mb + 1)])
                        wgu.append(a)
                    for ms in range(4):
                        mo = 4 * mb + ms
                        pg = psM.tile([128, OWN], F32, tag="pmm")
                        pu = psM.tile([128, OWN], F32, tag="pmm")
                        for k in range(8):
                            nc.tensor.matmul(
                                out=pg[:],
                                lhsT=wgu[k][:, 128 * ms:128 * (ms + 1)],
                                rhs=n2[k][:], start=(k == 0), stop=(k == 7))
                            nc.tensor.matmul(
                                out=pu[:],
                                lhsT=wgu[k][:, 512 + 128 * ms:512 + 128 * (ms + 1)],
                                rhs=n2[k][:], start=(k == 0), stop=(k == 7))
                        gs = sb.tile([128, OWN], BF, tag="gsilu")
                        nc.scalar.activation(out=gs[:], in_=pg[:], func=Silu)
                        nc.vector.tensor_tensor(out=ht[mo][:], in0=pu[:],
                                                in1=gs[:], op=MULT)
                # down-proj: two output chunks per pass, stream wd tiles
                ssq3 = psB.tile([1, OWN], F32, tag="psB")
                for mp in range(4):
                    pd0 = psM.tile([128, OWN], F32, tag="pmm")
                    pd1 = psM.tile([128, OWN], F32, tag="pmm")
                    for kk in range(32):
                        c_, j = divmod(kk, 4)
                        wt = wbig.tile([128, 256], BF, tag="wsm", bufs=4,
                                       name="wdt")
                        eng = nc.sync if kk % 2 == 0 else nc.scalar
                        eng.dma_start(
                            out=wt[:],
                            in_=ffn_g[l][c_, :, 8192 + 1024 * j + 256 * mp:
                                         8192 + 1024 * j + 256 * (mp + 1)])
                        nc.tensor.matmul(out=pd0[:], lhsT=wt[:, 0:128],
                                         rhs=ht[kk][:], start=(kk == 0),
                                         stop=(kk == 31))
                        nc.tensor.matmul(out=pd1[:], lhsT=wt[:, 128:256],
                                         rhs=ht[kk][:], start=(kk == 0),
                                         stop=(kk == 31))
                    nc.vector.tensor_tensor(out=x[2 * mp][:], in0=x[2 * mp][:],
                                            in1=pd0[:], op=ADD)
                    nc.vector.tensor_tensor(out=x[2 * mp + 1][:],
                                            in0=x[2 * mp + 1][:],
                                            in1=pd1[:], op=ADD)
                    ssq_acc(2 * mp, ssq3)
                    ssq_acc(2 * mp + 1, ssq3)
                nssq[0] = ssq3

            # ---- final norm -> nf (npool, survives body pools) ----
            nf = rmsnorm(ssq=nssq[0])
            for k in range(8):
                nc.sync.dma_start(out=nf_st[:, 512 * k:512 * (k + 1)],
                                  in_=nf[k][:])
            ag(nf_st[:], nf_g[:])

        # ---- vocab-sharded head: all tokens x our V/8 slice ----
        with ExitStack() as hd:
            hp = hd.enter_context(tc.tile_pool(name="hp", bufs=1))
            hw = hd.enter_context(tc.tile_pool(name="hw", bufs=2))
            whead = []
            for k in range(8):
                wt = hp.tile([128, VS], BF, tag=f"wh{k}", name=f"wh{k}")
                nc.sync.dma_start(out=wt[:], in_=woutc[128 * k:128 * (k + 1), :])
                whead.append(wt)
            for cp in range(C):
                nfo = hw.tile([128, 8 * OWN], BF, tag="nfo")
                nc.sync.dma_start(out=nfo[:], in_=nf_g[cp])
                for tb in range(4):
                    for vp in range(NVT // 2):
                        vt0, vt1 = 2 * vp, 2 * vp + 1
                        vw1 = min(512, VS - 512 * vt1)
                        ph0 = psM.tile([128, 512], F32, tag="pmm")
                        ph1 = psM.tile([128, 512], F32, tag="pmm")
                        for k in range(8):
                            lhs = nfo[:, 512 * k + 128 * tb:
                                      512 * k + 128 * (tb + 1)]
                            nc.tensor.matmul(
                                out=ph0[:], lhsT=lhs,
                                rhs=whead[k][:, 512 * vt0:512 * (vt0 + 1)],
                                start=(k == 0), stop=(k == 7))
                            nc.tensor.matmul(
                                out=ph1[:, :vw1], lhsT=lhs,
                                rhs=whead[k][:, 512 * vt1:512 * vt1 + vw1],
                                start=(k == 0), stop=(k == 7))
                        for vt, vw, ph, eng in ((vt0, 512, ph0, 0),
                                                (vt1, vw1, ph1, 1)):
                            ot = hw.tile([128, 512], F16, tag="hout")
                            if eng == 0:
                                nc.vector.tensor_copy(out=ot[:, :vw],
                                                      in_=ph[:, :vw])
                            else:
                                nc.scalar.activation(out=ot[:, :vw],
                                                     in_=ph[:, :vw],
                                                     func=Copy)
                            nc.sync.dma_start(
                                out=logits[512 * cp + 128 * tb:
                                           512 * cp + 128 * (tb + 1),
                                           512 * vt:512 * vt + vw],
                                in_=ot[:, :vw])

    return nc


# ------------------------------------------------------------------ host ---

_NC_CACHE = {}


def _get_nc():
    if "nc" not in _NC_CACHE:
        nc = build_nc()
        nc.finalize()
        _NC_CACHE["nc"] = nc
    return _NC_CACHE["nc"]


def _host_prep(inputs):
    inv_freq = 1.0 / ROPE_BASE ** (np.arange(0, HD, 2, dtype=np.float32) / HD)
    t = np.arange(S, dtype=np.float32)
    freqs = t[:, None] * inv_freq[None, :]
    ang = np.concatenate([freqs, freqs], axis=-1)       # [S, 64]
    cos_full, sin_full = np.cos(ang), np.sin(ang)
    cosT2 = np.empty((128, S), np.float32)
    sinT2 = np.empty((128, S), np.float32)
    for p in range(128):
        d = p % 64
        cosT2[p] = cos_full[:, d]
        sinT2[p] = sin_full[:, d] * (-1.0 if d < 32 else 1.0)

    R = np.zeros((128, 128), np.float32)
    for blk in range(2):
        o = blk * 64
        for j in range(32):
            R[o + 32 + j, o + j] = 1.0
            R[o + j, o + 32 + j] = 1.0

    naw = np.asarray(inputs["norm_attn_w"], np.float32)
    nfw = np.asarray(inputs["norm_ff_w"], np.float32)
    emb = np.asarray(inputs["token_emb"], np.float32)
    wq_ = (np.asarray(inputs["wq"], np.float32) * naw[:, :, None] * SCALE
           ).astype(BF16)
    wk_ = (np.asarray(inputs["wk"], np.float32) * naw[:, :, None]).astype(BF16)
    wv_ = (np.asarray(inputs["wv"], np.float32) * naw[:, :, None]).astype(BF16)
    wo_ = np.asarray(inputs["wo"], np.float32).astype(BF16)
    wg_ = (np.asarray(inputs["w_gate"], np.float32) * nfw[:, :, None]
           ).astype(BF16)
    wu_ = (np.asarray(inputs["w_up"], np.float32) * nfw[:, :, None]
           ).astype(BF16)
    wd_ = np.asarray(inputs["w_down"], np.float32).astype(BF16)
    wout_ = (np.asarray(inputs["w_out"], np.float32)
             * np.asarray(inputs["norm_final_w"], np.float32)[:, None]
             ).astype(BF16)
    rmat_b = np.ascontiguousarray(R.astype(BF16))

    idx_full = np.asarray(inputs["in_idx"]).astype(np.int64)
    tri = (np.arange(128)[:, None] <= np.arange(128)[None, :]).astype(np.float32)
    tri4 = np.tile(tri, (1, 4))
    in_maps = []
    for c in range(C):
        blks = own_blocks(c)
        rs = slice(128 * c, 128 * (c + 1))
        # own-token ids in column order (b, tt): (0,b0),(0,b1),(1,b0),(1,b1)
        ids = np.concatenate([idx_full[b, bl * BS:(bl + 1) * BS]
                              for b in range(B) for bl in blks])
        x0 = np.ascontiguousarray(
            emb[ids].T.reshape(8, 128, OWN).astype(BF16))
        pos = np.concatenate([np.arange(bl * BS, (bl + 1) * BS) for bl in blks])
        cosT = np.ascontiguousarray(
            np.concatenate([cosT2[:, pos], cosT2[:, pos]], axis=1).astype(BF16))
        sinT = np.ascontiguousarray(
            np.concatenate([sinT2[:, pos], sinT2[:, pos]], axis=1).astype(BF16))
        mk = np.zeros((16, 128, 512), np.float32)
        for t_, blk in enumerate(blks):
            for jj in range(8):
                j = jj if t_ == 0 else jj + 8
                if j < blk:
                    mk[8 * t_ + jj] = 1.0
                elif j == blk:
                    mk[8 * t_ + jj] = tri4
        qkv = np.ascontiguousarray(np.concatenate(
            [wk_[:, rs, :], wv_[:, rs, :], wq_[:, rs, :]], axis=2))
        wosh = np.ascontiguousarray(wo_[:, rs, :])
        ffn = np.empty((L, 128, 12288), BF16)
        for mb in range(8):
            ffn[:, :, 1024 * mb:1024 * mb + 512] = \
                wg_[:, rs, 512 * mb:512 * (mb + 1)]
            ffn[:, :, 1024 * mb + 512:1024 * (mb + 1)] = \
                wu_[:, rs, 512 * mb:512 * (mb + 1)]
        ffn[:, :, 8192:] = (
            wd_[:, 512 * c:512 * (c + 1), :]
            .reshape(L, 4, 128, D).transpose(0, 2, 1, 3).reshape(L, 128, 4096))
        in_maps.append({
            "x0": x0,
            "cosT": cosT,
            "sinT": sinT,
            "rmat": rmat_b,
            "masks": np.ascontiguousarray(mk.astype(BF16)),
            "qkv_sh": qkv,
            "wo_sh": wosh,
            "ffn_sh": np.ascontiguousarray(ffn),
            "woutc": np.ascontiguousarray(wout_[:, VS * c:VS * (c + 1)]),
        })
    return in_maps


def _assemble(results):
    out = np.empty((B, S, V), np.float32)
    for c in range(C):          # vocab-shard owner
        lg = np.asarray(results[c]["logits"]).astype(np.float32)
        for cp in range(C):     # token owner
            blks = own_blocks(cp)
            for b in range(B):
                for tt in range(2):
                    r0 = cp * 512 + 128 * (2 * b + tt)
                    out[b, blks[tt] * BS:(blks[tt] + 1) * BS,
                        VS * c:VS * (c + 1)] = lg[r0:r0 + 128]
    return out


def run(inputs, trace=False, trace_cores=None):
    nc = _get_nc()
    in_maps = _host_prep(inputs)
    res = run_bass_kernel_spmd(nc, in_maps, list(range(C)), trace=trace,
                               trace_cores=trace_cores)
    return _assemble(res.results), res


def kernel(**inputs):
    out, _ = run(inputs)
    return out
